# revision 43
# baseline (speedup 1.0000x reference)
"""3-layer GAT on Trainium2, 8 NeuronCores.

Strategy (dst-sharded, v2):
  - Nodes padded to NPAD (mult of 8*128); core c owns a contiguous range of
    NPC nodes.  All edges (incl. self-loops on every padded node) are routed
    to the core that owns their *destination*, sorted by dst, grouped into
    dst-blocks of 128 destination nodes, and padded to chunks of 128 edges.
  - Layer 1: x is replicated (tiny), so every core computes h = x @ W1 and
    the attention logits for ALL nodes locally and writes the combined rows
    [h (bf16), s|d (f32 tail)] to a core-local DRAM table -- no collective.
  - Layers 2/3: phase 1 runs on own nodes only; an AllGather replicates the
    combined rows to every core.
  - Attention logits s,d come from one tiny PE matmul per block against
    host-folded WA = [W @ a_src | W @ a_dst]  (s = h.a_src = y.(W a_src)).
  - Edge phase per dst-block: merged dma_gathers pull the combined rows of
    the edge sources (h[src], s[src]) and the 256B tails of the destinations
    (d[dst]).  Softmax numerator exp(leaky_relu(s+d) - c) is computed per
    edge (c = per-core scalar bound; softmax is shift-invariant so a
    per-core constant is exact since all edges of a dst live on one core).
    The weighted segment-sum over incoming edges is a PE matmul with a
    one-hot mask; for layers 1/2 exp is folded into per-head masks
    Smh = Sm * ex (halves the DVE volume vs scaling the gathered rows),
    for layer 3 (C=64) the gathered rows are scaled directly.  The
    denominator uses the raw mask with rhs = exp.  Skip connection
    (y @ lin_W + b) is a per-block PE matmul into PSUM; division, skip add,
    and ELU happen in the per-block epilogue; layer output is transposed
    (PE) into feat-major yT for the next layer's matmuls.
  - Layer 3: concat=False -> mean over 6 heads, no ELU; per-core rows DMA'd
    out, host concatenates and drops padding.
"""

import functools
import numpy as np
from contextlib import ExitStack

import ml_dtypes
import concourse.bass as bass
import concourse.bacc as bacc
import concourse.tile as tile
import concourse.masks as masks
from concourse import mybir
from concourse import library_config
from concourse._compat import cdiv

dt = mybir.dt
Alu = mybir.AluOpType
Act = mybir.ActivationFunctionType

BF16 = np.dtype(ml_dtypes.bfloat16)
NCORES = 8
P = 128

# layer configs: (F_in, F_out=H*C, H, C)
LAYERS = [
    (128, 1024, 4, 256),
    (1024, 1024, 4, 256),
    (1024, 384, 6, 64),
]
NEG_SLOPE = 0.2
OUT_DIM = 64
TAIL = 128  # tail units (bf16) appended to h in each combined row (256 B)


# ---------------------------------------------------------------------------
# host-side graph preprocessing
# ---------------------------------------------------------------------------

def _prep_graph(edge_index, n_pad):
    """Sort edges (plus self-loops on all padded nodes) by dst; bucket into
    dst-blocks of 128; pad each block's edge list to a globally uniform
    multiple of 128 (KMAX chunks, SPMD uniformity across cores)."""
    src = np.asarray(edge_index[0], dtype=np.int64)
    dst = np.asarray(edge_index[1], dtype=np.int64)
    loops = np.arange(n_pad, dtype=np.int64)
    src = np.concatenate([src, loops])
    dst = np.concatenate([dst, loops])

    order = np.argsort(dst, kind="stable")
    src, dst = src[order], dst[order]

    nblocks = n_pad // P  # global dst blocks
    blk = dst // P
    counts = np.bincount(blk, minlength=nblocks)
    kmax = int(cdiv(int(counts.max()), P))
    neb = kmax * P  # edges per block (padded)

    src_pad = np.zeros((nblocks, neb), dtype=np.int64)
    dst_pad = np.zeros((nblocks, neb), dtype=np.int64)
    valid = np.zeros((nblocks, neb), dtype=bool)
    starts = np.concatenate([[0], np.cumsum(counts)])
    for b in range(nblocks):
        c = counts[b]
        s0 = starts[b]
        src_pad[b, :c] = src[s0:s0 + c]
        dst_pad[b, :c] = dst[s0:s0 + c]
        valid[b, :c] = True

    # per-edge local dst index in e-partition-major layout [nblocks, 128, kmax]
    dst_local = (dst_pad - (np.arange(nblocks) * P)[:, None]).astype(np.int64)
    dst_local[~valid] = -1
    dl = dst_local.reshape(nblocks, kmax, P).transpose(0, 2, 1)
    dl = np.ascontiguousarray(dl.astype(np.int16))

    return dict(kmax=kmax, neb=neb, src_pad=src_pad, dst_pad=dst_pad, dl=dl)


def _wrap_idx(a):
    # [n] int -> [16, n//16] int16 (wrapped in 16 partitions; device replicates)
    n = a.shape[0]
    assert n % 16 == 0
    w = a.reshape(n // 16, 16).T.astype(np.int16)
    return np.ascontiguousarray(w)


# ---------------------------------------------------------------------------
# bass program builder
# ---------------------------------------------------------------------------

# per-layer combined-row dtype for the h part (messages); logits stay f32
COMB_DT = ["f8", "f8", "f8"]


def _comb_width(fo, H, esz):
    # row: [h (fo units) | s,d f32 (8H B) | pad to 256B multiple]; in units
    return cdiv(fo * esz + 8 * H, 256) * 256 // esz


def build_program(n_pad, kmax):
    npc = n_pad // NCORES      # nodes per core
    nb = npc // P              # dst blocks per core
    nbf = n_pad // P           # all dst blocks (layer-1 phase 1)
    ne = nb * kmax * P         # padded edges per core
    kq = cdiv(kmax, 4)         # gather piece size (chunks)
    pieces = [(k0, min(kq, kmax - k0)) for k0 in range(0, kmax, kq)]

    nc = bacc.Bacc("TRN2", target_bir_lowering=False, debug=False)

    f32, bf16, i16 = dt.float32, dt.bfloat16, dt.int16
    f8 = dt.float8e4
    i32 = dt.int32

    # ---------------- DRAM I/O ----------------
    xT = nc.dram_tensor("xT", [P, n_pad], bf16, kind="ExternalInput")
    xT_own = nc.dram_tensor("xT_own", [P, npc], bf16, kind="ExternalInput")
    W = []
    for li, (fi, fo, H, C) in enumerate(LAYERS):
        f_skip = OUT_DIM if li == 2 else fo
        W.append(dict(
            W=nc.dram_tensor(f"W{li}", [fi, fo], bf16, kind="ExternalInput"),
            linW=nc.dram_tensor(f"linW{li}", [fi, f_skip], bf16,
                                kind="ExternalInput"),
            brow=nc.dram_tensor(f"brow{li}", [1, f_skip], bf16,
                                kind="ExternalInput"),
            WA=nc.dram_tensor(f"WA{li}", [fi, 2 * H], bf16,
                              kind="ExternalInput"),
        ))
    idx_src = nc.dram_tensor("idx_src", [16, ne // 16], i16, kind="ExternalInput")
    idx_dst = nc.dram_tensor("idx_dst", [16, ne // 16], i16, kind="ExternalInput")
    idx_dstl = nc.dram_tensor("idx_dstl", [16, ne // 16], i16,
                              kind="ExternalInput")
    dstloc = nc.dram_tensor("dstloc", [P, nb * kmax], f32, kind="ExternalInput")
    out_dram = nc.dram_tensor("out", [npc, OUT_DIM], f32, kind="ExternalOutput")

    # combined-row tables are DECLARED bf16 (the collective stack's proven
    # byte-clean dtype); f8 layers interpret the same bytes via bitcast views
    comb_own, comb_full = [], []
    for li, (fi, fo, H, C) in enumerate(LAYERS):
        esz = 1 if COMB_DT[li] == "f8" else 2
        cw_st = _comb_width(fo, H, esz) * esz // 2   # width in bf16 units
        if li == 0:
            comb_own.append(None)
            comb_full.append(
                nc.dram_tensor(f"comb_full{li}", [n_pad, cw_st], bf16))
        else:
            comb_own.append(
                nc.dram_tensor(f"comb_own{li}", [npc, cw_st], bf16))
            comb_full.append(
                nc.dram_tensor(f"comb_full{li}", [n_pad, cw_st], bf16,
                               addr_space="Shared"))

    replica_groups = [list(range(NCORES))]

    with tile.TileContext(nc) as tc, ExitStack() as ctx:
        const_pool = ctx.enter_context(tc.tile_pool(name="const", bufs=1))
        wpool = ctx.enter_context(tc.tile_pool(name="w", bufs=1))
        hpool = ctx.enter_context(tc.tile_pool(name="h", bufs=3))
        gpool = ctx.enter_context(tc.tile_pool(name="g", bufs=6))
        tpool = ctx.enter_context(tc.tile_pool(name="t", bufs=2))
        mpool = ctx.enter_context(tc.tile_pool(name="m", bufs=3))
        epool = ctx.enter_context(tc.tile_pool(name="e", bufs=4))
        ypool = ctx.enter_context(tc.tile_pool(name="y", bufs=2))
        yTpool = ctx.enter_context(tc.tile_pool(name="yT", bufs=1))
        smallpool = ctx.enter_context(tc.tile_pool(name="small", bufs=4))
        psum_mm = ctx.enter_context(tc.tile_pool(name="psmm", bufs=3, space="PSUM"))
        psum_agg = ctx.enter_context(tc.tile_pool(name="psagg", bufs=3, space="PSUM"))
        psum_sm = ctx.enter_context(tc.tile_pool(name="pssm", bufs=2, space="PSUM"))

        nc.gpsimd.load_library(library_config.mlp)

        # constants
        ident = const_pool.tile([P, P], f32)
        masks.make_identity(nc, ident[:])
        ident_bf = const_pool.tile([P, P], bf16)
        nc.vector.tensor_copy(ident_bf[:], ident[:])
        ones_f32 = const_pool.tile([1, P], f32)
        nc.vector.memset(ones_f32[:], 1.0)
        ones_bf = const_pool.tile([1, P], bf16)
        nc.vector.memset(ones_bf[:], 1.0)

        # x resident for layer 1 (lhsT, bf16); full copy + own slice.
        # Loaded FIRST: phase 1 needs it immediately, while the index tiles
        # are only read by the edge phase much later.
        xT_sb = const_pool.tile([P, n_pad], bf16, tag="xT")
        nc.sync.dma_start(xT_sb[:], xT[:])
        xTo_sb = const_pool.tile([P, npc], bf16, tag="xTo")
        nc.sync.dma_start(xTo_sb[:], xT_own[:])

        # index tiles (persistent); replicate [16, C] -> [128, C] on device
        idxs_t = const_pool.tile([P, ne // 16], i16, tag="idxs")
        idxd_t = const_pool.tile([P, ne // 16], i16, tag="idxd")
        idxdl_t = const_pool.tile([P, ne // 16], i16, tag="idxdl")
        for gi in range(8):
            nc.scalar.dma_start(idxs_t[16 * gi:16 * (gi + 1), :], idx_src[:])
            nc.gpsimd.dma_start(idxd_t[16 * gi:16 * (gi + 1), :], idx_dst[:])
            nc.gpsimd.dma_start(idxdl_t[16 * gi:16 * (gi + 1), :],
                                idx_dstl[:])
        dstloc_t = const_pool.tile([P, nb * kmax], f32, tag="dstloc")
        nc.scalar.dma_start(dstloc_t[:], dstloc[:])
        iot32 = const_pool.tile([P, P], dt.int32, tag="iot32")
        nc.gpsimd.iota(iot32[:], pattern=[[1, P]], base=0, channel_multiplier=0)
        iot = const_pool.tile([P, P], bf16, tag="iot")
        nc.vector.tensor_copy(iot[:], iot32[:])

        # ---- per-layer derived params (units = comb dtype elements) ----
        def _params(li):
            fi, fo, H, C = LAYERS[li]
            esz = 1 if COMB_DT[li] == "f8" else 2
            cw = _comb_width(fo, H, esz)
            t_elem = 256 // esz   # tail-gather elem (256 B)
            t_off = cw - t_elem
            return dict(
                fi=fi, fo=fo, H=H, C=C, kin=fi // P, cw=cw, tail0=fo,
                esz=esz, cdt=f8 if esz == 1 else bf16,
                tsd=8 * H // esz, t_elem=t_elem, t_off=t_off,
                sf0=(fo - t_off) * esz // 4,  # f32 idx of s in tail gather
                f_skip=OUT_DIM if li == 2 else fo,
                segs=[(h0, min(512, fo - h0)) for h0 in range(0, fo, 512)])

        PR = [_params(li) for li in range(3)]
        lctx = {}   # per-layer phase-1 tiles: W, WA, smax
        yT_t = {}   # per-layer output yT tiles

        def load_p1_weights(li):
            pr = PR[li]
            W_sb = wpool.tile([P, pr["kin"], pr["fo"]], bf16, tag="W")
            nc.sync.dma_start(
                W_sb[:], W[li]["W"].rearrange("(k p) f -> p k f", p=P))
            WA_sb = wpool.tile([P, pr["kin"], 2 * pr["H"]], bf16, tag="WA")
            nc.sync.dma_start(
                WA_sb[:], W[li]["WA"].rearrange("(k p) j -> p k j", p=P))
            nblk = nbf if li == 0 else nb
            reds = smallpool.tile([P, 1, nblk], f32, tag=f"sx{li}")
            lctx[li] = dict(W=W_sb, WA=WA_sb, reds=reds, nblk=nblk)

        def phase1_block(li, b):
            """h = y@W + logits for one 128-node block; b is the global block
            id for L1 (x replicated), else the own-block id."""
            pr = PR[li]
            kin, H = pr["kin"], pr["H"]
            segs, cw, tail0, tsd = (pr["segs"], pr["cw"], pr["tail0"],
                                    pr["tsd"])
            full = li == 0

            def lhsf(k):
                if full:
                    return xT_sb[:, b * P:(b + 1) * P]
                return yT_t[li - 1][:, k, b * P:(b + 1) * P]

            ph = [psum_mm.tile([P, 512], f32, tag="mm", name=f"ph{li}_{si}")
                  for si in range(len(segs))]
            for si, (h0, hw_) in enumerate(segs):
                for k in range(kin):
                    nc.tensor.matmul(
                        ph[si][:, 0:hw_], lhsf(k),
                        lctx[li]["W"][:, k, h0:h0 + hw_],
                        start=(k == 0), stop=(k == kin - 1))
            # s/d logits: tiny matmul against folded WA
            psd = psum_sm.tile([P, 2 * H], f32, tag="sm", name=f"psd{li}",
                               padded_shape=[P, 512])
            for k in range(kin):
                nc.tensor.matmul(psd[:], lhsf(k), lctx[li]["WA"][:, k, :],
                                 start=(k == 0), stop=(k == kin - 1))
            # per-block joint max of s/d logits; reduced once per layer
            reds = lctx[li]["reds"]
            nc.vector.tensor_reduce(reds[:, 0, b:b + 1], psd[:, 0:2 * H],
                                    axis=mybir.AxisListType.X, op=Alu.max)
            # assemble comb row [h | s|d f32-bits]; write to DRAM
            hbf = hpool.tile([P, cw], pr["cdt"], tag="hbf")
            for si, (h0, hw_) in enumerate(segs):
                if full and si % 2 == 1:
                    nc.vector.tensor_copy(hbf[:, h0:h0 + hw_],
                                          ph[si][:, 0:hw_])
                else:
                    nc.scalar.activation(hbf[:, h0:h0 + hw_],
                                         ph[si][:, 0:hw_], Act.Copy)
            nc.scalar.activation(
                hbf[:, tail0:tail0 + tsd].bitcast(f32), psd[:], Act.Copy)
            tgt = comb_full[0] if full else comb_own[li]
            tgt_v = tgt[:].bitcast(pr["cdt"])
            qeng = (nc.sync, nc.scalar, nc.gpsimd)[b % 3] if full else nc.sync
            qeng.dma_start(tgt_v[b * P:(b + 1) * P, 0:tail0 + tsd],
                           hbf[:, 0:tail0 + tsd])

        dvt_t = {}

        def dvt_gather(li, b):
            """Pre-gather d[dst] tails for layer li's block b from comb_own
            (runs before the AllGather occupies the Pool queue)."""
            pr = PR[li]
            cdt_, esz_ = pr["cdt"], pr["esz"]
            cw_, t_elem_, t_off_ = pr["cw"], pr["t_elem"], pr["t_off"]
            sf0_, H_ = pr["sf0"], pr["H"]
            if b == 0:
                dvt_t[li] = epool.tile([P, nb, kmax, H_], f32,
                                       tag=f"dvt{li % 2}", bufs=1,
                                       name=f"dvt{li}")
            e0b = b * kmax * P
            T = tpool.tile([P, kmax, t_elem_], cdt_, tag="T")
            co_v = comb_own[li][:].bitcast(cdt_)
            nc.gpsimd.dma_gather(
                T[:].bitcast(i32),
                co_v[:, t_off_:t_off_ + t_elem_].bitcast(i32),
                idxdl_t[:, e0b // 16:(e0b + kmax * P) // 16],
                kmax * P, kmax * P, t_elem_ * esz_ // 4,
                elem_step=cw_ * esz_ // 4, single_packet=False)
            Tf_ = T[:].bitcast(f32)
            nc.vector.tensor_copy(
                dvt_t[li][:, b, :, :], Tf_[:, :, sf0_ + H_:sf0_ + 2 * H_])

        # ==== phase 1 of layer 1: every core computes ALL blocks ====
        load_p1_weights(0)
        for g in range(nbf):
            phase1_block(0, g)

        for li, (fi, fo, H, C) in enumerate(LAYERS):
            pr = PR[li]
            kin = pr["kin"]
            last = li == 2
            cdt = pr["cdt"]
            esz = pr["esz"]
            cw, tail0, tsd = pr["cw"], pr["tail0"], pr["tsd"]
            t_elem, t_off, sf0 = pr["t_elem"], pr["t_off"], pr["sf0"]
            f_skip = pr["f_skip"]
            segs = pr["segs"]
            full = li == 0
            fold = not last         # exp folded into per-head masks

            def lhs_own(k, b):
                if li == 0:
                    return xTo_sb[:, b * P:(b + 1) * P]
                return yT_t[li - 1][:, k, b * P:(b + 1) * P]

            # -------- edge-phase weights --------
            linW_sb = wpool.tile([P, kin, f_skip], bf16, tag="linW")
            nc.sync.dma_start(
                linW_sb[:], W[li]["linW"].rearrange("(k p) f -> p k f", p=P))
            brow_sb = wpool.tile([1, f_skip], bf16, tag="brow")
            nc.sync.dma_start(brow_sb[:], W[li]["brow"][:])
            # next layer's phase-1 weights (load overlaps this edge phase)
            if li < 2:
                load_p1_weights(li + 1)

            # -------- scalar logit bound c --------
            reds, nblk = lctx[li]["reds"], lctx[li]["nblk"]
            csum = smallpool.tile([P, 1], f32, tag="csum")
            nc.vector.tensor_reduce(csum[:], reds[:, 0, :],
                                    axis=mybir.AxisListType.X, op=Alu.max)
            nc.vector.tensor_scalar_mul(csum[:], csum[:], 2.0)
            ct = psum_sm.tile([1, P], f32, tag="sm", name="ct")
            nc.tensor.transpose(ct[:], csum[:], ident[:])
            c1 = smallpool.tile([1, 1], f32, tag="c1")
            nc.vector.tensor_reduce(c1[:], ct[:], axis=mybir.AxisListType.X,
                                    op=Alu.max)
            pc = psum_sm.tile([P, 1], f32, tag="sm", name="pc")
            nc.tensor.matmul(pc[:], ones_f32[:], c1[:], start=True, stop=True)
            ncP = smallpool.tile([P, 1], f32, tag="cP")
            nc.scalar.activation(ncP[:], pc[:], Act.Copy, scale=-1.0)

            # -------- T-gathers: d[dst] tails (own dst rows) --------
            # For layers 2/3 these read the core-local comb_own with local
            # dst ids and run inside the AllGather window.
            dvt = dvt_t.get(li)

            # -------- edge phase per dst block --------
            if not last:
                yT_t[li] = yTpool.tile([P, fo // P, npc], bf16,
                                       tag=f"yT{li % 2}", name=f"yT_new{li}")
            yT_new = yT_t.get(li)
            comb_ap = comb_full[li][:].bitcast(cdt)
            sksegs = [(h0, min(512, f_skip - h0)) for h0 in range(0, f_skip, 512)]
            for b in range(nb):
                e0b = b * kmax * P
                if full:
                    # d[dst] tails for the whole block: one gather
                    T = tpool.tile([P, kmax, t_elem], cdt, tag="T")
                    nc.gpsimd.dma_gather(
                        T[:].bitcast(i32),
                        comb_ap[:, t_off:t_off + t_elem].bitcast(i32),
                        idxd_t[:, e0b // 16:(e0b + kmax * P) // 16],
                        kmax * P, kmax * P, t_elem * esz // 4,
                        elem_step=cw * esz // 4, single_packet=False)
                    Tf = T[:].bitcast(f32)

                # skip GEMM for this block (PE, overlaps gathers)
                ps = [psum_mm.tile([P, 512], f32, tag="mm", name=f"ps{si}")
                      for si in range(len(sksegs))]
                for si, (h0, hw_) in enumerate(sksegs):
                    for k in range(kin):
                        nc.tensor.matmul(
                            ps[si][:, 0:hw_], lhs_own(k, b),
                            linW_sb[:, k, h0:h0 + hw_],
                            start=(k == 0), stop=False)
                    nc.tensor.matmul(
                        ps[si][:, 0:hw_], ones_bf[:],
                        brow_sb[:, h0:h0 + hw_],
                        start=False, stop=True)

                pagg = [psum_agg.tile([P, 512], f32, tag="pagg",
                                      name=f"pagg{si}")
                        for si in range(len(segs))]
                pden = psum_sm.tile([P, H], f32, tag="sm", name="pden",
                                    padded_shape=[P, 512])
                Gs = []
                for (k0, kh) in pieces:
                    e0 = (b * kmax + k0) * P
                    n_idx = kh * P
                    G = gpool.tile([P, kq, cw], cdt, tag="G")
                    nc.gpsimd.dma_gather(
                        G[:, 0:kh, :].bitcast(i32), comb_ap[:, :].bitcast(i32),
                        idxs_t[:, e0 // 16:(e0 + n_idx) // 16],
                        n_idx, n_idx, cw * esz // 4,
                        elem_step=cw * esz // 4, single_packet=False)
                    Gs.append(G)
                hpb = 512 // C  # heads per psum bank
                for pi, (k0, kh) in enumerate(pieces):
                    G = Gs[pi]
                    # alternate the piece's mask work between Pool and DVE
                    veng = nc.gpsimd if pi % 2 == 0 else nc.vector
                    Sm = mpool.tile([P, kq, P], cdt, tag="Sm")
                    for k in range(kh):
                        gk = b * kmax + k0 + k
                        veng.tensor_scalar(
                            Sm[:, k, :], iot[:], dstloc_t[:, gk:gk + 1], None,
                            op0=Alu.is_equal)

                    # logits -> exp(leaky(s+d) - c)  (e - c <= 0 since c is
                    # the global max bound, so no overflow clamp is needed;
                    # -c is folded into the exp bias)
                    sv = G[:, 0:kh,
                           tail0:tail0 + 4 * H // esz].bitcast(f32)
                    dv = (Tf[:, k0:k0 + kh, sf0 + H:sf0 + 2 * H] if full
                          else dvt[:, b, k0:k0 + kh, :])
                    ee = epool.tile([P, kq, H], f32, tag="ee")
                    nc.vector.tensor_add(ee[:, 0:kh, :], sv, dv)
                    nc.vector.scalar_tensor_tensor(
                        ee[:, 0:kh, :], ee[:, 0:kh, :], NEG_SLOPE,
                        ee[:, 0:kh, :], op0=Alu.mult, op1=Alu.max)
                    ex = epool.tile([P, kq, H], bf16, tag="ex")
                    nc.scalar.activation(ex[:, 0:kh, :], ee[:, 0:kh, :],
                                         Act.Exp, bias=ncP[:, 0:1])

                    hh2 = H // 2
                    if not last:
                        # per-head masks Smh = Sm * ex, split by head halves
                        # across Pool and DVE to shorten the critical link
                        Smh = mpool.tile([P, kq, H, P], bf16, tag="Smh")
                        for eng, ha, hbnd in ((nc.gpsimd, 0, hh2),
                                              (nc.vector, hh2, H)):
                            nh_ = hbnd - ha
                            eng.tensor_mul(
                                Smh[:, 0:kh, ha:hbnd],
                                Sm[:, 0:kh].unsqueeze(2).broadcast_to(
                                    [P, kh, nh_, P]),
                                ex[:, 0:kh, ha:hbnd].unsqueeze(3).broadcast_to(
                                    [P, kh, nh_, P]))
                        for k in range(kh):
                            kk = k0 + k
                            st, sp = kk == 0, kk == kmax - 1
                            for hh in range(H):
                                si, off = divmod(hh * C, 512)
                                bank_last = min((si + 1) * hpb, H) - 1
                                nc.tensor.matmul(
                                    pagg[si][:, off:off + C],
                                    Smh[:, k, hh, :],
                                    G[:, k, hh * C:(hh + 1) * C],
                                    start=st and hh % hpb == 0,
                                    stop=sp and hh == bank_last)
                            nc.tensor.matmul(pden[:], Sm[:, k, :],
                                             ex[:, k, :], start=st, stop=sp)
                    else:
                        # L3: scale rows into bf16 (fo < H*P), keep raw mask
                        Gb = mpool.tile([P, kq, fo], bf16, tag="Gb",
                                        bufs=2)
                        Gbv = Gb[:, 0:kh].rearrange("p k (h c) -> p k h c",
                                                    h=H)
                        Ghv = G[:, 0:kh, 0:fo].rearrange(
                            "p k (h c) -> p k h c", h=H)
                        for eng, ha, hbnd in ((nc.gpsimd, 0, hh2),
                                              (nc.vector, hh2, H)):
                            nh_ = hbnd - ha
                            eng.tensor_mul(
                                Gbv[:, :, ha:hbnd], Ghv[:, :, ha:hbnd],
                                ex[:, 0:kh, ha:hbnd].unsqueeze(3).broadcast_to(
                                    [P, kh, nh_, C]))
                        for k in range(kh):
                            kk = k0 + k
                            st, sp = kk == 0, kk == kmax - 1
                            nc.tensor.matmul(pagg[0][:, 0:fo], Sm[:, k, :],
                                             Gb[:, k, :], start=st, stop=sp)
                            nc.tensor.matmul(pden[:], Sm[:, k, :],
                                             ex[:, k, :], start=st, stop=sp)

                # epilogue for block b
                rden = smallpool.tile([P, H], f32, tag="rden")
                nc.vector.reciprocal(rden[:], pden[:])
                yf = ypool.tile([P, fo], f32, tag="yf")
                if not last:
                    for si, (h0, hw_) in enumerate(segs):
                        nh = hw_ // C
                        hh0 = h0 // C
                        nc.vector.tensor_mul(
                            yf[:, h0:h0 + hw_].rearrange(
                                "p (h c) -> p h c", h=nh),
                            pagg[si][:, 0:hw_].rearrange(
                                "p (h c) -> p h c", h=nh),
                            rden[:, hh0:hh0 + nh].unsqueeze(2)
                                .broadcast_to([P, nh, C]))
                    for si, (h0, hw_) in enumerate(sksegs):
                        nc.vector.tensor_add(yf[:, h0:h0 + hw_],
                                             yf[:, h0:h0 + hw_],
                                             ps[si][:, 0:hw_])
                    # ELU: y = max(yf,0) + exp(min(yf,0)) - 1
                    mn = ypool.tile([P, fo], f32, tag="mn", bufs=1)
                    nc.gpsimd.tensor_scalar_min(mn[:], yf[:], 0.0)
                    nc.scalar.activation(mn[:], mn[:], Act.Exp)
                    nc.vector.scalar_tensor_tensor(
                        yf[:], yf[:], 0.0, mn[:], op0=Alu.max, op1=Alu.add)
                    nc.scalar.activation(yf[:], yf[:], Act.Copy, bias=-1.0)
                    # transpose into yT_new (4 transposes per psum bank,
                    # one Act copy per bank)
                    for j0 in range(0, fo // P, 4):
                        jn = min(4, fo // P - j0)
                        pt = psum_sm.tile([P, 4, P], f32, tag="sm", name="pt",
                                          padded_shape=[P, 4, P])
                        for j in range(jn):
                            nc.tensor.transpose(
                                pt[:, j, :], yf[:, (j0 + j) * P:(j0 + j + 1) * P],
                                ident[:])
                        nc.scalar.activation(
                            yT_new[:, j0:j0 + jn, b * P:(b + 1) * P],
                            pt[:, 0:jn, :], Act.Copy)
                    # next layer's phase 1 for this block (pipelined so the
                    # AllGather input is ready as soon as the loop ends)
                    phase1_block(li + 1, b)
                    dvt_gather(li + 1, b)
                else:
                    for si, (h0, hw_) in enumerate(segs):
                        nh = hw_ // C
                        hh0 = h0 // C
                        nc.vector.tensor_mul(
                            yf[:, h0:h0 + hw_].rearrange(
                                "p (h c) -> p h c", h=nh),
                            pagg[si][:, 0:hw_].rearrange(
                                "p (h c) -> p h c", h=nh),
                            rden[:, hh0:hh0 + nh].unsqueeze(2)
                                .broadcast_to([P, nh, C]))
                    # mean over heads + skip
                    yo = ypool.tile([P, OUT_DIM], f32, tag="yo")
                    nc.vector.tensor_reduce(
                        yo[:], yf[:].rearrange("p (h c) -> p c h", h=H),
                        axis=mybir.AxisListType.X, op=Alu.add)
                    nc.vector.tensor_scalar_mul(yo[:], yo[:], 1.0 / H)
                    nc.vector.tensor_add(yo[:], yo[:], ps[0][:, 0:OUT_DIM])
                    nc.sync.dma_start(out_dram[b * P:(b + 1) * P, :], yo[:])

            # -------- AllGather next layer's combined rows --------
            if li < 2:
                nc.gpsimd.collective_compute(
                    "AllGather", Alu.bypass, replica_groups=replica_groups,
                    ins=[comb_own[li + 1][:]], outs=[comb_full[li + 1][:]])

    nc.compile()
    return nc


# ---------------------------------------------------------------------------
# host wrapper
# ---------------------------------------------------------------------------

@functools.lru_cache(maxsize=2)
def _cached_program(n_pad, kmax):
    return build_program(n_pad, kmax)


def make_in_maps(x, edge_index, weights):
    """weights: list of 3 dicts with keys W, linW, brow, aS, aD (numpy f32)."""
    n = x.shape[0]
    n_pad = cdiv(n, NCORES * P) * NCORES * P
    npc = n_pad // NCORES
    nb = npc // P

    g = _prep_graph(edge_index, n_pad)

    x_pad = np.zeros((n_pad, x.shape[1]), np.float32)
    x_pad[:n] = np.asarray(x, np.float32)
    xT_all = np.ascontiguousarray(x_pad.T.astype(BF16))

    layer_w = []
    for li, lw in enumerate(weights):
        Wf = np.asarray(lw["W"], np.float64)
        aS = np.asarray(lw["aS"], np.float64)   # [H, C]
        aD = np.asarray(lw["aD"], np.float64)
        H, C = aS.shape
        fo = H * C
        # WA[k, h] = sum_c W[k, h*C+c] * a[h, c]
        Wr = Wf.reshape(-1, H, C)
        WAs = np.einsum("khc,hc->kh", Wr, aS)
        WAd = np.einsum("khc,hc->kh", Wr, aD)
        WA = np.concatenate([WAs, WAd], axis=1)  # [fi, 2H]
        layer_w.append(dict(
            W=np.ascontiguousarray(Wf.astype(BF16)),
            linW=np.ascontiguousarray(
                np.asarray(lw["linW"], np.float32).astype(BF16)),
            brow=np.ascontiguousarray(
                np.asarray(lw["brow"], np.float32).astype(BF16)[None, :]),
            WA=np.ascontiguousarray(WA.astype(BF16)),
        ))

    in_maps = []
    for c in range(NCORES):
        blo, bhi = c * nb, (c + 1) * nb
        nbc = bhi - blo
        kmax = g["kmax"]
        m = dict(
            xT=xT_all,
            xT_own=np.ascontiguousarray(xT_all[:, c * npc:(c + 1) * npc]),
            idx_src=_wrap_idx(g["src_pad"][blo:bhi].reshape(-1)),
            idx_dst=_wrap_idx(g["dst_pad"][blo:bhi].reshape(-1)),
            idx_dstl=_wrap_idx(np.maximum(
                g["dst_pad"][blo:bhi].reshape(-1) - c * npc, 0)),
            dstloc=np.ascontiguousarray(
                g["dl"][blo:bhi].transpose(1, 0, 2).reshape(P, nbc * kmax)
                .astype(np.float32)),
        )
        for li, lw in enumerate(layer_w):
            for key in ("W", "linW", "brow", "WA"):
                m[f"{key}{li}"] = lw[key]
        in_maps.append(m)
    return in_maps, g, n_pad


def _weights_from_kwargs(W1, a1_src, a1_dst, b1, lin1_W, lin1_b,
                         W2, a2_src, a2_dst, b2, lin2_W, lin2_b,
                         W3, a3_src, a3_dst, b3, lin3_W, lin3_b):
    return [
        dict(W=W1, linW=lin1_W, brow=np.asarray(b1) + np.asarray(lin1_b),
             aS=a1_src, aD=a1_dst),
        dict(W=W2, linW=lin2_W, brow=np.asarray(b2) + np.asarray(lin2_b),
             aS=a2_src, aD=a2_dst),
        dict(W=W3, linW=lin3_W, brow=np.asarray(b3) + np.asarray(lin3_b),
             aS=a3_src, aD=a3_dst),
    ]


def run_gat(inputs, trace=False, **run_kwargs):
    from concourse.bass_utils import run_bass_kernel_spmd

    kw = {k: inputs[k] for k in (
        "W1", "a1_src", "a1_dst", "b1", "lin1_W", "lin1_b",
        "W2", "a2_src", "a2_dst", "b2", "lin2_W", "lin2_b",
        "W3", "a3_src", "a3_dst", "b3", "lin3_W", "lin3_b")}
    weights = _weights_from_kwargs(**kw)
    x, edge_index = inputs["x"], inputs["edge_index"]
    in_maps, g, n_pad = make_in_maps(x, edge_index, weights)
    nc = _cached_program(n_pad, g["kmax"])
    res = run_bass_kernel_spmd(nc, in_maps, list(range(NCORES)),
                               trace=trace, **run_kwargs)
    out = np.concatenate([res.results[c]["out"] for c in range(NCORES)],
                         axis=0)
    n = x.shape[0]
    return np.ascontiguousarray(out[:n]).astype(np.float32), res


def kernel(**inputs):
    return run_gat(inputs)[0]



# revision 44
# speedup vs baseline: 1.1024x; 1.1024x over previous
"""3-layer GAT on Trainium2, 8 NeuronCores.

Strategy (dst-sharded, v2):
  - Nodes padded to NPAD (mult of 8*128); core c owns a contiguous range of
    NPC nodes.  All edges (incl. self-loops on every padded node) are routed
    to the core that owns their *destination*, sorted by dst, grouped into
    dst-blocks of 128 destination nodes, and padded to chunks of 128 edges.
  - Layer 1: x is replicated (tiny), so every core computes h = x @ W1 and
    the attention logits for ALL nodes locally and writes the combined rows
    [h (bf16), s|d (f32 tail)] to a core-local DRAM table -- no collective.
  - Layers 2/3: phase 1 runs on own nodes only; an AllGather replicates the
    combined rows to every core.
  - Attention logits s,d come from one tiny PE matmul per block against
    host-folded WA = [W @ a_src | W @ a_dst]  (s = h.a_src = y.(W a_src)).
  - Edge phase per dst-block: merged dma_gathers pull the combined rows of
    the edge sources (h[src], s[src]) and the 256B tails of the destinations
    (d[dst]).  Softmax numerator exp(leaky_relu(s+d) - c) is computed per
    edge (c = per-core scalar bound; softmax is shift-invariant so a
    per-core constant is exact since all edges of a dst live on one core).
    The weighted segment-sum over incoming edges is a PE matmul with a
    one-hot mask; for layers 1/2 exp is folded into per-head masks
    Smh = Sm * ex (halves the DVE volume vs scaling the gathered rows),
    for layer 3 (C=64) the gathered rows are scaled directly.  The
    denominator uses the raw mask with rhs = exp.  Skip connection
    (y @ lin_W + b) is a per-block PE matmul into PSUM; division, skip add,
    and ELU happen in the per-block epilogue; layer output is transposed
    (PE) into feat-major yT for the next layer's matmuls.
  - Layer 3: concat=False -> mean over 6 heads, no ELU; per-core rows DMA'd
    out, host concatenates and drops padding.
"""

import functools
import numpy as np
from contextlib import ExitStack

import ml_dtypes
import concourse.bass as bass
import concourse.bacc as bacc
import concourse.tile as tile
import concourse.masks as masks
from concourse import mybir
from concourse import library_config
from concourse._compat import cdiv

dt = mybir.dt
Alu = mybir.AluOpType
Act = mybir.ActivationFunctionType

BF16 = np.dtype(ml_dtypes.bfloat16)
NCORES = 8
P = 128

# layer configs: (F_in, F_out=H*C, H, C)
LAYERS = [
    (128, 1024, 4, 256),
    (1024, 1024, 4, 256),
    (1024, 384, 6, 64),
]
NEG_SLOPE = 0.2
OUT_DIM = 64
TAIL = 128  # tail units (bf16) appended to h in each combined row (256 B)


# ---------------------------------------------------------------------------
# host-side graph preprocessing
# ---------------------------------------------------------------------------

def _prep_graph(edge_index, n_pad):
    """Sort edges (plus self-loops on all padded nodes) by dst; bucket into
    dst-blocks of 128; pad each block's edge list to a globally uniform
    multiple of 128 (KMAX chunks, SPMD uniformity across cores)."""
    src = np.asarray(edge_index[0], dtype=np.int64)
    dst = np.asarray(edge_index[1], dtype=np.int64)
    loops = np.arange(n_pad, dtype=np.int64)
    src = np.concatenate([src, loops])
    dst = np.concatenate([dst, loops])

    order = np.argsort(dst, kind="stable")
    src, dst = src[order], dst[order]

    nblocks = n_pad // P  # global dst blocks
    blk = dst // P
    counts = np.bincount(blk, minlength=nblocks)
    kmax = int(cdiv(int(counts.max()), P))
    neb = kmax * P  # edges per block (padded)

    src_pad = np.zeros((nblocks, neb), dtype=np.int64)
    dst_pad = np.zeros((nblocks, neb), dtype=np.int64)
    valid = np.zeros((nblocks, neb), dtype=bool)
    starts = np.concatenate([[0], np.cumsum(counts)])
    for b in range(nblocks):
        c = counts[b]
        s0 = starts[b]
        src_pad[b, :c] = src[s0:s0 + c]
        dst_pad[b, :c] = dst[s0:s0 + c]
        valid[b, :c] = True

    # per-edge local dst index in e-partition-major layout [nblocks, 128, kmax]
    dst_local = (dst_pad - (np.arange(nblocks) * P)[:, None]).astype(np.int64)
    dst_local[~valid] = -1
    dl = dst_local.reshape(nblocks, kmax, P).transpose(0, 2, 1)
    dl = np.ascontiguousarray(dl.astype(np.int16))

    return dict(kmax=kmax, neb=neb, src_pad=src_pad, dst_pad=dst_pad, dl=dl)


def _wrap_idx(a):
    # [n] int -> [16, n//16] int16 (wrapped in 16 partitions; device replicates)
    n = a.shape[0]
    assert n % 16 == 0
    w = a.reshape(n // 16, 16).T.astype(np.int16)
    return np.ascontiguousarray(w)


# ---------------------------------------------------------------------------
# bass program builder
# ---------------------------------------------------------------------------

# per-layer combined-row dtype for the h part (messages); logits stay f32
COMB_DT = ["f8", "f8", "f8"]


def _comb_width(fo, H, esz):
    # row: [h (fo units) | s,d f32 (8H B) | pad to 256B multiple]; in units
    return cdiv(fo * esz + 8 * H, 256) * 256 // esz


def build_program(n_pad, kmax):
    npc = n_pad // NCORES      # nodes per core
    nb = npc // P              # dst blocks per core
    nbf = n_pad // P           # all dst blocks (layer-1 phase 1)
    ne = nb * kmax * P         # padded edges per core
    kq = cdiv(kmax, 4)         # gather piece size (chunks)
    pieces = [(k0, min(kq, kmax - k0)) for k0 in range(0, kmax, kq)]

    nc = bacc.Bacc("TRN2", target_bir_lowering=False, debug=False)

    f32, bf16, i16 = dt.float32, dt.bfloat16, dt.int16
    f8 = dt.float8e4
    i32 = dt.int32

    # ---------------- DRAM I/O ----------------
    xT = nc.dram_tensor("xT", [P, n_pad], bf16, kind="ExternalInput")
    xT_own = nc.dram_tensor("xT_own", [P, npc], bf16, kind="ExternalInput")
    W = []
    for li, (fi, fo, H, C) in enumerate(LAYERS):
        f_skip = OUT_DIM if li == 2 else fo
        W.append(dict(
            W=nc.dram_tensor(f"W{li}", [fi, fo], bf16, kind="ExternalInput"),
            linW=nc.dram_tensor(f"linW{li}", [fi, f_skip], bf16,
                                kind="ExternalInput"),
            brow=nc.dram_tensor(f"brow{li}", [1, f_skip], bf16,
                                kind="ExternalInput"),
            WA=nc.dram_tensor(f"WA{li}", [fi, 2 * H], bf16,
                              kind="ExternalInput"),
        ))
    idx_src = nc.dram_tensor("idx_src", [16, ne // 16], i16, kind="ExternalInput")
    idx_dst = nc.dram_tensor("idx_dst", [16, ne // 16], i16, kind="ExternalInput")
    idx_dstl = nc.dram_tensor("idx_dstl", [16, ne // 16], i16,
                              kind="ExternalInput")
    dstloc = nc.dram_tensor("dstloc", [P, nb * kmax], f32, kind="ExternalInput")
    out_dram = nc.dram_tensor("out", [npc, OUT_DIM], f32, kind="ExternalOutput")

    # combined-row tables are DECLARED bf16 (the collective stack's proven
    # byte-clean dtype); f8 layers interpret the same bytes via bitcast views
    comb_own, comb_full = [], []
    for li, (fi, fo, H, C) in enumerate(LAYERS):
        esz = 1 if COMB_DT[li] == "f8" else 2
        cw_st = _comb_width(fo, H, esz) * esz // 2   # width in bf16 units
        if li == 0:
            comb_own.append(None)
            comb_full.append(
                nc.dram_tensor(f"comb_full{li}", [n_pad, cw_st], bf16))
        else:
            comb_own.append(
                nc.dram_tensor(f"comb_own{li}", [npc, cw_st], bf16))
            comb_full.append(
                nc.dram_tensor(f"comb_full{li}", [n_pad, cw_st], bf16,
                               addr_space="Shared"))

    replica_groups = [list(range(NCORES))]

    with tile.TileContext(nc) as tc, ExitStack() as ctx:
        const_pool = ctx.enter_context(tc.tile_pool(name="const", bufs=1))
        wpool = ctx.enter_context(tc.tile_pool(name="w", bufs=1))
        hpool = ctx.enter_context(tc.tile_pool(name="h", bufs=3))
        gpool = ctx.enter_context(tc.tile_pool(name="g", bufs=6))
        tpool = ctx.enter_context(tc.tile_pool(name="t", bufs=2))
        mpool = ctx.enter_context(tc.tile_pool(name="m", bufs=3))
        epool = ctx.enter_context(tc.tile_pool(name="e", bufs=4))
        ypool = ctx.enter_context(tc.tile_pool(name="y", bufs=2))
        yTpool = ctx.enter_context(tc.tile_pool(name="yT", bufs=1))
        smallpool = ctx.enter_context(tc.tile_pool(name="small", bufs=4))
        psum_mm = ctx.enter_context(tc.tile_pool(name="psmm", bufs=3, space="PSUM"))
        psum_agg = ctx.enter_context(tc.tile_pool(name="psagg", bufs=3, space="PSUM"))
        psum_sm = ctx.enter_context(tc.tile_pool(name="pssm", bufs=2, space="PSUM"))

        nc.gpsimd.load_library(library_config.mlp)

        # constants
        ident = const_pool.tile([P, P], f32)
        masks.make_identity(nc, ident[:])
        ident_bf = const_pool.tile([P, P], bf16)
        nc.vector.tensor_copy(ident_bf[:], ident[:])
        ones_f32 = const_pool.tile([1, P], f32)
        nc.vector.memset(ones_f32[:], 1.0)
        ones_bf = const_pool.tile([1, P], bf16)
        nc.vector.memset(ones_bf[:], 1.0)

        # x resident for layer 1 (lhsT, bf16); full copy + own slice.
        # Loaded FIRST: phase 1 needs it immediately, while the index tiles
        # are only read by the edge phase much later.
        xT_sb = const_pool.tile([P, n_pad], bf16, tag="xT")
        nc.sync.dma_start(xT_sb[:], xT[:])
        xTo_sb = const_pool.tile([P, npc], bf16, tag="xTo")
        nc.sync.dma_start(xTo_sb[:], xT_own[:])

        # index tiles (persistent); replicate [16, C] -> [128, C] on device
        idxs_t = const_pool.tile([P, ne // 16], i16, tag="idxs")
        idxd_t = const_pool.tile([P, ne // 16], i16, tag="idxd")
        idxdl_t = const_pool.tile([P, ne // 16], i16, tag="idxdl")
        for gi in range(8):
            nc.scalar.dma_start(idxs_t[16 * gi:16 * (gi + 1), :], idx_src[:])
            nc.gpsimd.dma_start(idxd_t[16 * gi:16 * (gi + 1), :], idx_dst[:])
            nc.gpsimd.dma_start(idxdl_t[16 * gi:16 * (gi + 1), :],
                                idx_dstl[:])
        dstloc_t = const_pool.tile([P, nb * kmax], f32, tag="dstloc")
        nc.scalar.dma_start(dstloc_t[:], dstloc[:])
        iot32 = const_pool.tile([P, P], dt.int32, tag="iot32")
        nc.gpsimd.iota(iot32[:], pattern=[[1, P]], base=0, channel_multiplier=0)
        iot = const_pool.tile([P, P], bf16, tag="iot")
        nc.vector.tensor_copy(iot[:], iot32[:])

        # ---- per-layer derived params (units = comb dtype elements) ----
        def _params(li):
            fi, fo, H, C = LAYERS[li]
            esz = 1 if COMB_DT[li] == "f8" else 2
            cw = _comb_width(fo, H, esz)
            t_elem = 256 // esz   # tail-gather elem (256 B)
            t_off = cw - t_elem
            return dict(
                fi=fi, fo=fo, H=H, C=C, kin=fi // P, cw=cw, tail0=fo,
                esz=esz, cdt=f8 if esz == 1 else bf16,
                tsd=8 * H // esz, t_elem=t_elem, t_off=t_off,
                sf0=(fo - t_off) * esz // 4,  # f32 idx of s in tail gather
                f_skip=OUT_DIM if li == 2 else fo,
                segs=[(h0, min(512, fo - h0)) for h0 in range(0, fo, 512)])

        PR = [_params(li) for li in range(3)]
        lctx = {}   # per-layer phase-1 tiles: W, WA, smax
        yT_t = {}   # per-layer output yT tiles

        def load_p1_weights(li):
            pr = PR[li]
            W_sb = wpool.tile([P, pr["kin"], pr["fo"]], bf16, tag="W")
            nc.sync.dma_start(
                W_sb[:], W[li]["W"].rearrange("(k p) f -> p k f", p=P))
            WA_sb = wpool.tile([P, pr["kin"], 2 * pr["H"]], bf16, tag="WA")
            nc.sync.dma_start(
                WA_sb[:], W[li]["WA"].rearrange("(k p) j -> p k j", p=P))
            nblk = nbf if li == 0 else nb
            reds = smallpool.tile([P, 1, nblk], f32, tag=f"sx{li}")
            lctx[li] = dict(W=W_sb, WA=WA_sb, reds=reds, nblk=nblk)

        def phase1_block(li, b):
            """h = y@W + logits for one 128-node block; b is the global block
            id for L1 (x replicated), else the own-block id."""
            pr = PR[li]
            kin, H = pr["kin"], pr["H"]
            segs, cw, tail0, tsd = (pr["segs"], pr["cw"], pr["tail0"],
                                    pr["tsd"])
            full = li == 0

            def lhsf(k):
                if full:
                    return xT_sb[:, b * P:(b + 1) * P]
                return yT_t[li - 1][:, k, b * P:(b + 1) * P]

            ph = [psum_mm.tile([P, 512], f32, tag="mm", name=f"ph{li}_{si}")
                  for si in range(len(segs))]
            for si, (h0, hw_) in enumerate(segs):
                for k in range(kin):
                    nc.tensor.matmul(
                        ph[si][:, 0:hw_], lhsf(k),
                        lctx[li]["W"][:, k, h0:h0 + hw_],
                        start=(k == 0), stop=(k == kin - 1))
            # s/d logits: tiny matmul against folded WA
            psd = psum_sm.tile([P, 2 * H], f32, tag="sm", name=f"psd{li}",
                               padded_shape=[P, 512])
            for k in range(kin):
                nc.tensor.matmul(psd[:], lhsf(k), lctx[li]["WA"][:, k, :],
                                 start=(k == 0), stop=(k == kin - 1))
            # per-block joint max of s/d logits; reduced once per layer
            reds = lctx[li]["reds"]
            nc.vector.tensor_reduce(reds[:, 0, b:b + 1], psd[:, 0:2 * H],
                                    axis=mybir.AxisListType.X, op=Alu.max)
            # assemble comb row [h | s|d f32-bits]; write to DRAM
            hbf = hpool.tile([P, cw], pr["cdt"], tag="hbf")
            for si, (h0, hw_) in enumerate(segs):
                if full and si % 2 == 1:
                    nc.vector.tensor_copy(hbf[:, h0:h0 + hw_],
                                          ph[si][:, 0:hw_])
                else:
                    nc.scalar.activation(hbf[:, h0:h0 + hw_],
                                         ph[si][:, 0:hw_], Act.Copy)
            nc.scalar.activation(
                hbf[:, tail0:tail0 + tsd].bitcast(f32), psd[:], Act.Copy)
            tgt = comb_full[0] if full else comb_own[li]
            tgt_v = tgt[:].bitcast(pr["cdt"])
            qeng = (nc.sync, nc.scalar, nc.gpsimd)[b % 3] if full else nc.sync
            qeng.dma_start(tgt_v[b * P:(b + 1) * P, 0:tail0 + tsd],
                           hbf[:, 0:tail0 + tsd])

        dvt_t = {}

        def dvt_gather(li, b):
            """Pre-gather d[dst] tails for layer li's block b from comb_own
            (runs before the AllGather occupies the Pool queue)."""
            pr = PR[li]
            cdt_, esz_ = pr["cdt"], pr["esz"]
            cw_, t_elem_, t_off_ = pr["cw"], pr["t_elem"], pr["t_off"]
            sf0_, H_ = pr["sf0"], pr["H"]
            if b == 0:
                dvt_t[li] = epool.tile([P, nb, kmax, H_], f32,
                                       tag=f"dvt{li % 2}", bufs=1,
                                       name=f"dvt{li}")
            e0b = b * kmax * P
            T = tpool.tile([P, kmax, t_elem_], cdt_, tag="T")
            co_v = comb_own[li][:].bitcast(cdt_)
            nc.gpsimd.dma_gather(
                T[:].bitcast(i32),
                co_v[:, t_off_:t_off_ + t_elem_].bitcast(i32),
                idxdl_t[:, e0b // 16:(e0b + kmax * P) // 16],
                kmax * P, kmax * P, t_elem_ * esz_ // 4,
                elem_step=cw_ * esz_ // 4, single_packet=False)
            Tf_ = T[:].bitcast(f32)
            nc.vector.tensor_copy(
                dvt_t[li][:, b, :, :], Tf_[:, :, sf0_ + H_:sf0_ + 2 * H_])

        # ==== phase 1 of layer 1: every core computes ALL blocks ====
        load_p1_weights(0)
        for g in range(nbf):
            phase1_block(0, g)

        for li, (fi, fo, H, C) in enumerate(LAYERS):
            pr = PR[li]
            kin = pr["kin"]
            last = li == 2
            cdt = pr["cdt"]
            esz = pr["esz"]
            cw, tail0, tsd = pr["cw"], pr["tail0"], pr["tsd"]
            t_elem, t_off, sf0 = pr["t_elem"], pr["t_off"], pr["sf0"]
            f_skip = pr["f_skip"]
            segs = pr["segs"]
            full = li == 0
            fold = not last         # exp folded into per-head masks

            def lhs_own(k, b):
                if li == 0:
                    return xTo_sb[:, b * P:(b + 1) * P]
                return yT_t[li - 1][:, k, b * P:(b + 1) * P]

            # -------- edge-phase weights --------
            linW_sb = wpool.tile([P, kin, f_skip], bf16, tag="linW")
            nc.sync.dma_start(
                linW_sb[:], W[li]["linW"].rearrange("(k p) f -> p k f", p=P))
            brow_sb = wpool.tile([1, f_skip], bf16, tag="brow")
            nc.sync.dma_start(brow_sb[:], W[li]["brow"][:])
            # next layer's phase-1 weights (load overlaps this edge phase)
            if li < 2:
                load_p1_weights(li + 1)

            # -------- scalar logit bound c --------
            reds, nblk = lctx[li]["reds"], lctx[li]["nblk"]
            csum = smallpool.tile([P, 1], f32, tag="csum")
            nc.vector.tensor_reduce(csum[:], reds[:, 0, :],
                                    axis=mybir.AxisListType.X, op=Alu.max)
            nc.vector.tensor_scalar_mul(csum[:], csum[:], 2.0)
            ct = psum_sm.tile([1, P], f32, tag="sm", name="ct")
            nc.tensor.transpose(ct[:], csum[:], ident[:])
            c1 = smallpool.tile([1, 1], f32, tag="c1")
            nc.vector.tensor_reduce(c1[:], ct[:], axis=mybir.AxisListType.X,
                                    op=Alu.max)
            pc = psum_sm.tile([P, 1], f32, tag="sm", name="pc")
            nc.tensor.matmul(pc[:], ones_f32[:], c1[:], start=True, stop=True)
            ncP = smallpool.tile([P, 1], f32, tag="cP")
            nc.scalar.activation(ncP[:], pc[:], Act.Copy, scale=-1.0)

            # -------- T-gathers: d[dst] tails (own dst rows) --------
            # For layers 2/3 these read the core-local comb_own with local
            # dst ids and run inside the AllGather window.
            if not full:
                for b in range(nb):
                    dvt_gather(li, b)
            dvt = dvt_t.get(li)

            # -------- edge phase per dst block --------
            if not last:
                yT_t[li] = yTpool.tile([P, fo // P, npc], bf16,
                                       tag=f"yT{li % 2}", name=f"yT_new{li}")
            yT_new = yT_t.get(li)
            comb_ap = comb_full[li][:].bitcast(cdt)
            sksegs = [(h0, min(512, f_skip - h0)) for h0 in range(0, f_skip, 512)]
            for b in range(nb):
                e0b = b * kmax * P
                if full:
                    # d[dst] tails for the whole block: one gather
                    T = tpool.tile([P, kmax, t_elem], cdt, tag="T")
                    nc.gpsimd.dma_gather(
                        T[:].bitcast(i32),
                        comb_ap[:, t_off:t_off + t_elem].bitcast(i32),
                        idxd_t[:, e0b // 16:(e0b + kmax * P) // 16],
                        kmax * P, kmax * P, t_elem * esz // 4,
                        elem_step=cw * esz // 4, single_packet=False)
                    Tf = T[:].bitcast(f32)

                # skip GEMM for this block (PE, overlaps gathers)
                ps = [psum_mm.tile([P, 512], f32, tag="mm", name=f"ps{si}")
                      for si in range(len(sksegs))]
                for si, (h0, hw_) in enumerate(sksegs):
                    for k in range(kin):
                        nc.tensor.matmul(
                            ps[si][:, 0:hw_], lhs_own(k, b),
                            linW_sb[:, k, h0:h0 + hw_],
                            start=(k == 0), stop=False)
                    nc.tensor.matmul(
                        ps[si][:, 0:hw_], ones_bf[:],
                        brow_sb[:, h0:h0 + hw_],
                        start=False, stop=True)

                pagg = [psum_agg.tile([P, 512], f32, tag="pagg",
                                      name=f"pagg{si}")
                        for si in range(len(segs))]
                pden = psum_sm.tile([P, H], f32, tag="sm", name="pden",
                                    padded_shape=[P, 512])
                Gs = []
                for (k0, kh) in pieces:
                    e0 = (b * kmax + k0) * P
                    n_idx = kh * P
                    G = gpool.tile([P, kq, cw], cdt, tag="G")
                    nc.gpsimd.dma_gather(
                        G[:, 0:kh, :].bitcast(i32), comb_ap[:, :].bitcast(i32),
                        idxs_t[:, e0 // 16:(e0 + n_idx) // 16],
                        n_idx, n_idx, cw * esz // 4,
                        elem_step=cw * esz // 4, single_packet=False)
                    Gs.append(G)
                hpb = 512 // C  # heads per psum bank
                for pi, (k0, kh) in enumerate(pieces):
                    G = Gs[pi]
                    # alternate the piece's mask work between Pool and DVE
                    veng = nc.gpsimd if pi % 2 == 0 else nc.vector
                    Sm = mpool.tile([P, kq, P], cdt, tag="Sm")
                    for k in range(kh):
                        gk = b * kmax + k0 + k
                        veng.tensor_scalar(
                            Sm[:, k, :], iot[:], dstloc_t[:, gk:gk + 1], None,
                            op0=Alu.is_equal)

                    # logits -> exp(leaky(s+d) - c)  (e - c <= 0 since c is
                    # the global max bound, so no overflow clamp is needed;
                    # -c is folded into the exp bias)
                    sv = G[:, 0:kh,
                           tail0:tail0 + 4 * H // esz].bitcast(f32)
                    dv = (Tf[:, k0:k0 + kh, sf0 + H:sf0 + 2 * H] if full
                          else dvt[:, b, k0:k0 + kh, :])
                    ee = epool.tile([P, kq, H], f32, tag="ee")
                    nc.vector.tensor_add(ee[:, 0:kh, :], sv, dv)
                    nc.vector.scalar_tensor_tensor(
                        ee[:, 0:kh, :], ee[:, 0:kh, :], NEG_SLOPE,
                        ee[:, 0:kh, :], op0=Alu.mult, op1=Alu.max)
                    ex = epool.tile([P, kq, H], bf16, tag="ex")
                    nc.scalar.activation(ex[:, 0:kh, :], ee[:, 0:kh, :],
                                         Act.Exp, bias=ncP[:, 0:1])

                    hh2 = H // 2
                    if not last:
                        # per-head masks Smh = Sm * ex, split by head halves
                        # across Pool and DVE to shorten the critical link
                        Smh = mpool.tile([P, kq, H, P], bf16, tag="Smh")
                        for eng, ha, hbnd in ((nc.gpsimd, 0, hh2),
                                              (nc.vector, hh2, H)):
                            nh_ = hbnd - ha
                            eng.tensor_mul(
                                Smh[:, 0:kh, ha:hbnd],
                                Sm[:, 0:kh].unsqueeze(2).broadcast_to(
                                    [P, kh, nh_, P]),
                                ex[:, 0:kh, ha:hbnd].unsqueeze(3).broadcast_to(
                                    [P, kh, nh_, P]))
                        for k in range(kh):
                            kk = k0 + k
                            st, sp = kk == 0, kk == kmax - 1
                            for hh in range(H):
                                si, off = divmod(hh * C, 512)
                                bank_last = min((si + 1) * hpb, H) - 1
                                nc.tensor.matmul(
                                    pagg[si][:, off:off + C],
                                    Smh[:, k, hh, :],
                                    G[:, k, hh * C:(hh + 1) * C],
                                    start=st and hh % hpb == 0,
                                    stop=sp and hh == bank_last)
                            nc.tensor.matmul(pden[:], Sm[:, k, :],
                                             ex[:, k, :], start=st, stop=sp)
                    else:
                        # L3: scale rows into bf16 (fo < H*P), keep raw mask
                        Gb = mpool.tile([P, kq, fo], bf16, tag="Gb",
                                        bufs=2)
                        Gbv = Gb[:, 0:kh].rearrange("p k (h c) -> p k h c",
                                                    h=H)
                        Ghv = G[:, 0:kh, 0:fo].rearrange(
                            "p k (h c) -> p k h c", h=H)
                        for eng, ha, hbnd in ((nc.gpsimd, 0, hh2),
                                              (nc.vector, hh2, H)):
                            nh_ = hbnd - ha
                            eng.tensor_mul(
                                Gbv[:, :, ha:hbnd], Ghv[:, :, ha:hbnd],
                                ex[:, 0:kh, ha:hbnd].unsqueeze(3).broadcast_to(
                                    [P, kh, nh_, C]))
                        for k in range(kh):
                            kk = k0 + k
                            st, sp = kk == 0, kk == kmax - 1
                            nc.tensor.matmul(pagg[0][:, 0:fo], Sm[:, k, :],
                                             Gb[:, k, :], start=st, stop=sp)
                            nc.tensor.matmul(pden[:], Sm[:, k, :],
                                             ex[:, k, :], start=st, stop=sp)

                # epilogue for block b
                rden = smallpool.tile([P, H], f32, tag="rden")
                nc.vector.reciprocal(rden[:], pden[:])
                yf = ypool.tile([P, fo], f32, tag="yf")
                if not last:
                    for si, (h0, hw_) in enumerate(segs):
                        nh = hw_ // C
                        hh0 = h0 // C
                        nc.vector.tensor_mul(
                            yf[:, h0:h0 + hw_].rearrange(
                                "p (h c) -> p h c", h=nh),
                            pagg[si][:, 0:hw_].rearrange(
                                "p (h c) -> p h c", h=nh),
                            rden[:, hh0:hh0 + nh].unsqueeze(2)
                                .broadcast_to([P, nh, C]))
                    for si, (h0, hw_) in enumerate(sksegs):
                        nc.vector.tensor_add(yf[:, h0:h0 + hw_],
                                             yf[:, h0:h0 + hw_],
                                             ps[si][:, 0:hw_])
                    # ELU: y = max(yf,0) + exp(min(yf,0)) - 1
                    mn = ypool.tile([P, fo], f32, tag="mn", bufs=1)
                    nc.gpsimd.tensor_scalar_min(mn[:], yf[:], 0.0)
                    nc.scalar.activation(mn[:], mn[:], Act.Exp)
                    nc.vector.scalar_tensor_tensor(
                        yf[:], yf[:], 0.0, mn[:], op0=Alu.max, op1=Alu.add)
                    nc.scalar.activation(yf[:], yf[:], Act.Copy, bias=-1.0)
                    # transpose into yT_new (4 transposes per psum bank,
                    # one Act copy per bank)
                    for j0 in range(0, fo // P, 4):
                        jn = min(4, fo // P - j0)
                        pt = psum_sm.tile([P, 4, P], f32, tag="sm", name="pt",
                                          padded_shape=[P, 4, P])
                        for j in range(jn):
                            nc.tensor.transpose(
                                pt[:, j, :], yf[:, (j0 + j) * P:(j0 + j + 1) * P],
                                ident[:])
                        nc.scalar.activation(
                            yT_new[:, j0:j0 + jn, b * P:(b + 1) * P],
                            pt[:, 0:jn, :], Act.Copy)
                    # next layer's phase 1 for this block (pipelined so the
                    # AllGather input is ready as soon as the loop ends)
                    phase1_block(li + 1, b)
                else:
                    for si, (h0, hw_) in enumerate(segs):
                        nh = hw_ // C
                        hh0 = h0 // C
                        nc.vector.tensor_mul(
                            yf[:, h0:h0 + hw_].rearrange(
                                "p (h c) -> p h c", h=nh),
                            pagg[si][:, 0:hw_].rearrange(
                                "p (h c) -> p h c", h=nh),
                            rden[:, hh0:hh0 + nh].unsqueeze(2)
                                .broadcast_to([P, nh, C]))
                    # mean over heads + skip
                    yo = ypool.tile([P, OUT_DIM], f32, tag="yo")
                    nc.vector.tensor_reduce(
                        yo[:], yf[:].rearrange("p (h c) -> p c h", h=H),
                        axis=mybir.AxisListType.X, op=Alu.add)
                    nc.vector.tensor_scalar_mul(yo[:], yo[:], 1.0 / H)
                    nc.vector.tensor_add(yo[:], yo[:], ps[0][:, 0:OUT_DIM])
                    nc.sync.dma_start(out_dram[b * P:(b + 1) * P, :], yo[:])

            # -------- AllGather next layer's combined rows --------
            if li < 2:
                nc.gpsimd.collective_compute(
                    "AllGather", Alu.bypass, replica_groups=replica_groups,
                    ins=[comb_own[li + 1][:]], outs=[comb_full[li + 1][:]])

    nc.compile()
    return nc


# ---------------------------------------------------------------------------
# host wrapper
# ---------------------------------------------------------------------------

@functools.lru_cache(maxsize=2)
def _cached_program(n_pad, kmax):
    return build_program(n_pad, kmax)


def make_in_maps(x, edge_index, weights):
    """weights: list of 3 dicts with keys W, linW, brow, aS, aD (numpy f32)."""
    n = x.shape[0]
    n_pad = cdiv(n, NCORES * P) * NCORES * P
    npc = n_pad // NCORES
    nb = npc // P

    g = _prep_graph(edge_index, n_pad)

    x_pad = np.zeros((n_pad, x.shape[1]), np.float32)
    x_pad[:n] = np.asarray(x, np.float32)
    xT_all = np.ascontiguousarray(x_pad.T.astype(BF16))

    layer_w = []
    for li, lw in enumerate(weights):
        Wf = np.asarray(lw["W"], np.float64)
        aS = np.asarray(lw["aS"], np.float64)   # [H, C]
        aD = np.asarray(lw["aD"], np.float64)
        H, C = aS.shape
        fo = H * C
        # WA[k, h] = sum_c W[k, h*C+c] * a[h, c]
        Wr = Wf.reshape(-1, H, C)
        WAs = np.einsum("khc,hc->kh", Wr, aS)
        WAd = np.einsum("khc,hc->kh", Wr, aD)
        WA = np.concatenate([WAs, WAd], axis=1)  # [fi, 2H]
        layer_w.append(dict(
            W=np.ascontiguousarray(Wf.astype(BF16)),
            linW=np.ascontiguousarray(
                np.asarray(lw["linW"], np.float32).astype(BF16)),
            brow=np.ascontiguousarray(
                np.asarray(lw["brow"], np.float32).astype(BF16)[None, :]),
            WA=np.ascontiguousarray(WA.astype(BF16)),
        ))

    in_maps = []
    for c in range(NCORES):
        blo, bhi = c * nb, (c + 1) * nb
        nbc = bhi - blo
        kmax = g["kmax"]
        m = dict(
            xT=xT_all,
            xT_own=np.ascontiguousarray(xT_all[:, c * npc:(c + 1) * npc]),
            idx_src=_wrap_idx(g["src_pad"][blo:bhi].reshape(-1)),
            idx_dst=_wrap_idx(g["dst_pad"][blo:bhi].reshape(-1)),
            idx_dstl=_wrap_idx(np.maximum(
                g["dst_pad"][blo:bhi].reshape(-1) - c * npc, 0)),
            dstloc=np.ascontiguousarray(
                g["dl"][blo:bhi].transpose(1, 0, 2).reshape(P, nbc * kmax)
                .astype(np.float32)),
        )
        for li, lw in enumerate(layer_w):
            for key in ("W", "linW", "brow", "WA"):
                m[f"{key}{li}"] = lw[key]
        in_maps.append(m)
    return in_maps, g, n_pad


def _weights_from_kwargs(W1, a1_src, a1_dst, b1, lin1_W, lin1_b,
                         W2, a2_src, a2_dst, b2, lin2_W, lin2_b,
                         W3, a3_src, a3_dst, b3, lin3_W, lin3_b):
    return [
        dict(W=W1, linW=lin1_W, brow=np.asarray(b1) + np.asarray(lin1_b),
             aS=a1_src, aD=a1_dst),
        dict(W=W2, linW=lin2_W, brow=np.asarray(b2) + np.asarray(lin2_b),
             aS=a2_src, aD=a2_dst),
        dict(W=W3, linW=lin3_W, brow=np.asarray(b3) + np.asarray(lin3_b),
             aS=a3_src, aD=a3_dst),
    ]


def run_gat(inputs, trace=False, **run_kwargs):
    from concourse.bass_utils import run_bass_kernel_spmd

    kw = {k: inputs[k] for k in (
        "W1", "a1_src", "a1_dst", "b1", "lin1_W", "lin1_b",
        "W2", "a2_src", "a2_dst", "b2", "lin2_W", "lin2_b",
        "W3", "a3_src", "a3_dst", "b3", "lin3_W", "lin3_b")}
    weights = _weights_from_kwargs(**kw)
    x, edge_index = inputs["x"], inputs["edge_index"]
    in_maps, g, n_pad = make_in_maps(x, edge_index, weights)
    nc = _cached_program(n_pad, g["kmax"])
    res = run_bass_kernel_spmd(nc, in_maps, list(range(NCORES)),
                               trace=trace, **run_kwargs)
    out = np.concatenate([res.results[c]["out"] for c in range(NCORES)],
                         axis=0)
    n = x.shape[0]
    return np.ascontiguousarray(out[:n]).astype(np.float32), res


def kernel(**inputs):
    return run_gat(inputs)[0]



# revision 45
# speedup vs baseline: 1.1173x; 1.0136x over previous
"""3-layer GAT on Trainium2, 8 NeuronCores.

Strategy (dst-sharded, v2):
  - Nodes padded to NPAD (mult of 8*128); core c owns a contiguous range of
    NPC nodes.  All edges (incl. self-loops on every padded node) are routed
    to the core that owns their *destination*, sorted by dst, grouped into
    dst-blocks of 128 destination nodes, and padded to chunks of 128 edges.
  - Layer 1: x is replicated (tiny), so every core computes h = x @ W1 and
    the attention logits for ALL nodes locally and writes the combined rows
    [h (bf16), s|d (f32 tail)] to a core-local DRAM table -- no collective.
  - Layers 2/3: phase 1 runs on own nodes only; an AllGather replicates the
    combined rows to every core.
  - Attention logits s,d come from one tiny PE matmul per block against
    host-folded WA = [W @ a_src | W @ a_dst]  (s = h.a_src = y.(W a_src)).
  - Edge phase per dst-block: merged dma_gathers pull the combined rows of
    the edge sources (h[src], s[src]) and the 256B tails of the destinations
    (d[dst]).  Softmax numerator exp(leaky_relu(s+d) - c) is computed per
    edge (c = per-core scalar bound; softmax is shift-invariant so a
    per-core constant is exact since all edges of a dst live on one core).
    The weighted segment-sum over incoming edges is a PE matmul with a
    one-hot mask; for layers 1/2 exp is folded into per-head masks
    Smh = Sm * ex (halves the DVE volume vs scaling the gathered rows),
    for layer 3 (C=64) the gathered rows are scaled directly.  The
    denominator uses the raw mask with rhs = exp.  Skip connection
    (y @ lin_W + b) is a per-block PE matmul into PSUM; division, skip add,
    and ELU happen in the per-block epilogue; layer output is transposed
    (PE) into feat-major yT for the next layer's matmuls.
  - Layer 3: concat=False -> mean over 6 heads, no ELU; per-core rows DMA'd
    out, host concatenates and drops padding.
"""

import functools
import numpy as np
from contextlib import ExitStack

import ml_dtypes
import concourse.bass as bass
import concourse.bacc as bacc
import concourse.tile as tile
import concourse.masks as masks
from concourse import mybir
from concourse import library_config
from concourse._compat import cdiv

dt = mybir.dt
Alu = mybir.AluOpType
Act = mybir.ActivationFunctionType

BF16 = np.dtype(ml_dtypes.bfloat16)
NCORES = 8
P = 128

# layer configs: (F_in, F_out=H*C, H, C)
LAYERS = [
    (128, 1024, 4, 256),
    (1024, 1024, 4, 256),
    (1024, 384, 6, 64),
]
NEG_SLOPE = 0.2
OUT_DIM = 64
TAIL = 128  # tail units (bf16) appended to h in each combined row (256 B)


# ---------------------------------------------------------------------------
# host-side graph preprocessing
# ---------------------------------------------------------------------------

def _prep_graph(edge_index, n_pad):
    """Sort edges (plus self-loops on all padded nodes) by dst; bucket into
    dst-blocks of 128; pad each block's edge list to a globally uniform
    multiple of 128 (KMAX chunks, SPMD uniformity across cores)."""
    src = np.asarray(edge_index[0], dtype=np.int64)
    dst = np.asarray(edge_index[1], dtype=np.int64)
    loops = np.arange(n_pad, dtype=np.int64)
    src = np.concatenate([src, loops])
    dst = np.concatenate([dst, loops])

    order = np.argsort(dst, kind="stable")
    src, dst = src[order], dst[order]

    nblocks = n_pad // P  # global dst blocks
    blk = dst // P
    counts = np.bincount(blk, minlength=nblocks)
    kmax = int(cdiv(int(counts.max()), P))
    neb = kmax * P  # edges per block (padded)

    src_pad = np.zeros((nblocks, neb), dtype=np.int64)
    dst_pad = np.zeros((nblocks, neb), dtype=np.int64)
    valid = np.zeros((nblocks, neb), dtype=bool)
    starts = np.concatenate([[0], np.cumsum(counts)])
    for b in range(nblocks):
        c = counts[b]
        s0 = starts[b]
        src_pad[b, :c] = src[s0:s0 + c]
        dst_pad[b, :c] = dst[s0:s0 + c]
        valid[b, :c] = True

    # per-edge local dst index in e-partition-major layout [nblocks, 128, kmax]
    dst_local = (dst_pad - (np.arange(nblocks) * P)[:, None]).astype(np.int64)
    dst_local[~valid] = -1
    dl = dst_local.reshape(nblocks, kmax, P).transpose(0, 2, 1)
    dl = np.ascontiguousarray(dl.astype(np.int16))

    return dict(kmax=kmax, neb=neb, src_pad=src_pad, dst_pad=dst_pad, dl=dl)


def _wrap_idx(a):
    # [n] int -> [16, n//16] int16 (wrapped in 16 partitions; device replicates)
    n = a.shape[0]
    assert n % 16 == 0
    w = a.reshape(n // 16, 16).T.astype(np.int16)
    return np.ascontiguousarray(w)


# ---------------------------------------------------------------------------
# bass program builder
# ---------------------------------------------------------------------------

# per-layer combined-row dtype for the h part (messages); logits stay f32
COMB_DT = ["f8", "f8", "f8"]


def _comb_width(fo, H, esz):
    # row: [h (fo units) | s,d f32 (8H B) | pad to 256B multiple]; in units
    return cdiv(fo * esz + 8 * H, 256) * 256 // esz


def build_program(n_pad, kmax):
    npc = n_pad // NCORES      # nodes per core
    nb = npc // P              # dst blocks per core
    nbf = n_pad // P           # all dst blocks (layer-1 phase 1)
    ne = nb * kmax * P         # padded edges per core
    kq = cdiv(kmax, 4)         # gather piece size (chunks)
    pieces = [(k0, min(kq, kmax - k0)) for k0 in range(0, kmax, kq)]

    nc = bacc.Bacc("TRN2", target_bir_lowering=False, debug=False)

    f32, bf16, i16 = dt.float32, dt.bfloat16, dt.int16
    f8 = dt.float8e4
    i32 = dt.int32

    # ---------------- DRAM I/O ----------------
    xT = nc.dram_tensor("xT", [P, n_pad], bf16, kind="ExternalInput")
    xT_own = nc.dram_tensor("xT_own", [P, npc], bf16, kind="ExternalInput")
    W = []
    for li, (fi, fo, H, C) in enumerate(LAYERS):
        f_skip = OUT_DIM if li == 2 else fo
        W.append(dict(
            W=nc.dram_tensor(f"W{li}", [fi, fo], bf16, kind="ExternalInput"),
            linW=nc.dram_tensor(f"linW{li}", [fi, f_skip], bf16,
                                kind="ExternalInput"),
            brow=nc.dram_tensor(f"brow{li}", [1, f_skip], bf16,
                                kind="ExternalInput"),
            WA=nc.dram_tensor(f"WA{li}", [fi, 2 * H], bf16,
                              kind="ExternalInput"),
        ))
    idx_src = nc.dram_tensor("idx_src", [16, ne // 16], i16, kind="ExternalInput")
    idx_dst = nc.dram_tensor("idx_dst", [16, ne // 16], i16, kind="ExternalInput")
    idx_dstl = nc.dram_tensor("idx_dstl", [16, ne // 16], i16,
                              kind="ExternalInput")
    dstloc = nc.dram_tensor("dstloc", [P, nb * kmax], f32, kind="ExternalInput")
    out_dram = nc.dram_tensor("out", [npc, OUT_DIM], f32, kind="ExternalOutput")

    # combined-row tables are DECLARED bf16 (the collective stack's proven
    # byte-clean dtype); f8 layers interpret the same bytes via bitcast views
    comb_own, comb_full = [], []
    for li, (fi, fo, H, C) in enumerate(LAYERS):
        esz = 1 if COMB_DT[li] == "f8" else 2
        cw_st = _comb_width(fo, H, esz) * esz // 2   # width in bf16 units
        if li == 0:
            comb_own.append(None)
            comb_full.append(
                nc.dram_tensor(f"comb_full{li}", [n_pad, cw_st], bf16))
        else:
            comb_own.append(
                nc.dram_tensor(f"comb_own{li}", [npc, cw_st], bf16))
            comb_full.append(
                nc.dram_tensor(f"comb_full{li}", [n_pad, cw_st], bf16,
                               addr_space="Shared"))

    replica_groups = [list(range(NCORES))]

    with tile.TileContext(nc) as tc, ExitStack() as ctx:
        const_pool = ctx.enter_context(tc.tile_pool(name="const", bufs=1))
        wpool = ctx.enter_context(tc.tile_pool(name="w", bufs=1))
        hpool = ctx.enter_context(tc.tile_pool(name="h", bufs=3))
        gpool = ctx.enter_context(tc.tile_pool(name="g", bufs=6))
        tpool = ctx.enter_context(tc.tile_pool(name="t", bufs=2))
        mpool = ctx.enter_context(tc.tile_pool(name="m", bufs=3))
        epool = ctx.enter_context(tc.tile_pool(name="e", bufs=4))
        ypool = ctx.enter_context(tc.tile_pool(name="y", bufs=2))
        yTpool = ctx.enter_context(tc.tile_pool(name="yT", bufs=1))
        smallpool = ctx.enter_context(tc.tile_pool(name="small", bufs=4))
        psum_mm = ctx.enter_context(tc.tile_pool(name="psmm", bufs=3, space="PSUM"))
        psum_agg = ctx.enter_context(tc.tile_pool(name="psagg", bufs=3, space="PSUM"))
        psum_sm = ctx.enter_context(tc.tile_pool(name="pssm", bufs=2, space="PSUM"))

        nc.gpsimd.load_library(library_config.mlp)

        # constants
        ident = const_pool.tile([P, P], f32)
        masks.make_identity(nc, ident[:])
        ident_bf = const_pool.tile([P, P], bf16)
        nc.vector.tensor_copy(ident_bf[:], ident[:])
        ones_f32 = const_pool.tile([1, P], f32)
        nc.vector.memset(ones_f32[:], 1.0)
        ones_bf = const_pool.tile([1, P], bf16)
        nc.vector.memset(ones_bf[:], 1.0)

        # x resident for layer 1 (lhsT, bf16); full copy + own slice.
        # Loaded FIRST: phase 1 needs it immediately, while the index tiles
        # are only read by the edge phase much later.
        xT_sb = const_pool.tile([P, n_pad], bf16, tag="xT")
        nc.sync.dma_start(xT_sb[:], xT[:])
        xTo_sb = const_pool.tile([P, npc], bf16, tag="xTo")
        nc.sync.dma_start(xTo_sb[:], xT_own[:])

        # index tiles (persistent); replicate [16, C] -> [128, C] on device
        idxs_t = const_pool.tile([P, ne // 16], i16, tag="idxs")
        idxd_t = const_pool.tile([P, ne // 16], i16, tag="idxd")
        idxdl_t = const_pool.tile([P, ne // 16], i16, tag="idxdl")
        for gi in range(8):
            nc.scalar.dma_start(idxs_t[16 * gi:16 * (gi + 1), :], idx_src[:])
            nc.gpsimd.dma_start(idxd_t[16 * gi:16 * (gi + 1), :], idx_dst[:])
            nc.gpsimd.dma_start(idxdl_t[16 * gi:16 * (gi + 1), :],
                                idx_dstl[:])
        dstloc_t = const_pool.tile([P, nb * kmax], f32, tag="dstloc")
        nc.scalar.dma_start(dstloc_t[:], dstloc[:])
        iot32 = const_pool.tile([P, P], dt.int32, tag="iot32")
        nc.gpsimd.iota(iot32[:], pattern=[[1, P]], base=0, channel_multiplier=0)
        iot = const_pool.tile([P, P], bf16, tag="iot")
        nc.vector.tensor_copy(iot[:], iot32[:])

        # ---- per-layer derived params (units = comb dtype elements) ----
        def _params(li):
            fi, fo, H, C = LAYERS[li]
            esz = 1 if COMB_DT[li] == "f8" else 2
            cw = _comb_width(fo, H, esz)
            t_elem = 256 // esz   # tail-gather elem (256 B)
            t_off = cw - t_elem
            return dict(
                fi=fi, fo=fo, H=H, C=C, kin=fi // P, cw=cw, tail0=fo,
                esz=esz, cdt=f8 if esz == 1 else bf16,
                tsd=8 * H // esz, t_elem=t_elem, t_off=t_off,
                sf0=(fo - t_off) * esz // 4,  # f32 idx of s in tail gather
                f_skip=OUT_DIM if li == 2 else fo,
                segs=[(h0, min(512, fo - h0)) for h0 in range(0, fo, 512)])

        PR = [_params(li) for li in range(3)]
        lctx = {}   # per-layer phase-1 tiles: W, WA, smax
        yT_t = {}   # per-layer output yT tiles

        def load_p1_weights(li):
            pr = PR[li]
            W_sb = wpool.tile([P, pr["kin"], pr["fo"]], bf16, tag="W")
            nc.sync.dma_start(
                W_sb[:], W[li]["W"].rearrange("(k p) f -> p k f", p=P))
            WA_sb = wpool.tile([P, pr["kin"], 2 * pr["H"]], bf16, tag="WA")
            nc.sync.dma_start(
                WA_sb[:], W[li]["WA"].rearrange("(k p) j -> p k j", p=P))
            nblk = nbf if li == 0 else nb
            reds = smallpool.tile([P, 1, nblk], f32, tag=f"sx{li}")
            lctx[li] = dict(W=W_sb, WA=WA_sb, reds=reds, nblk=nblk)

        def phase1_block(li, b):
            """h = y@W + logits for one 128-node block; b is the global block
            id for L1 (x replicated), else the own-block id."""
            pr = PR[li]
            kin, H = pr["kin"], pr["H"]
            segs, cw, tail0, tsd = (pr["segs"], pr["cw"], pr["tail0"],
                                    pr["tsd"])
            full = li == 0

            def lhsf(k):
                if full:
                    return xT_sb[:, b * P:(b + 1) * P]
                return yT_t[li - 1][:, k, b * P:(b + 1) * P]

            ph = [psum_mm.tile([P, 512], f32, tag="mm", name=f"ph{li}_{si}")
                  for si in range(len(segs))]
            for si, (h0, hw_) in enumerate(segs):
                for k in range(kin):
                    nc.tensor.matmul(
                        ph[si][:, 0:hw_], lhsf(k),
                        lctx[li]["W"][:, k, h0:h0 + hw_],
                        start=(k == 0), stop=(k == kin - 1))
            # s/d logits: tiny matmul against folded WA
            psd = psum_sm.tile([P, 2 * H], f32, tag="sm", name=f"psd{li}",
                               padded_shape=[P, 512])
            for k in range(kin):
                nc.tensor.matmul(psd[:], lhsf(k), lctx[li]["WA"][:, k, :],
                                 start=(k == 0), stop=(k == kin - 1))
            # per-block joint max of s/d logits; reduced once per layer
            reds = lctx[li]["reds"]
            nc.vector.tensor_reduce(reds[:, 0, b:b + 1], psd[:, 0:2 * H],
                                    axis=mybir.AxisListType.X, op=Alu.max)
            # assemble comb row [h | s|d f32-bits]; write to DRAM
            hbf = hpool.tile([P, cw], pr["cdt"], tag="hbf")
            for si, (h0, hw_) in enumerate(segs):
                if full and si % 2 == 1:
                    nc.vector.tensor_copy(hbf[:, h0:h0 + hw_],
                                          ph[si][:, 0:hw_])
                else:
                    nc.scalar.activation(hbf[:, h0:h0 + hw_],
                                         ph[si][:, 0:hw_], Act.Copy)
            nc.scalar.activation(
                hbf[:, tail0:tail0 + tsd].bitcast(f32), psd[:], Act.Copy)
            tgt = comb_full[0] if full else comb_own[li]
            tgt_v = tgt[:].bitcast(pr["cdt"])
            qeng = (nc.sync, nc.gpsimd)[b % 2] if full else nc.sync
            qeng.dma_start(tgt_v[b * P:(b + 1) * P, 0:tail0 + tsd],
                           hbf[:, 0:tail0 + tsd])

        dvt_t = {}

        def dvt_gather(li, b):
            """Pre-gather d[dst] tails for layer li's block b from comb_own
            (runs before the AllGather occupies the Pool queue)."""
            pr = PR[li]
            cdt_, esz_ = pr["cdt"], pr["esz"]
            cw_, t_elem_, t_off_ = pr["cw"], pr["t_elem"], pr["t_off"]
            sf0_, H_ = pr["sf0"], pr["H"]
            if b == 0:
                dvt_t[li] = epool.tile([P, nb, kmax, H_], f32,
                                       tag=f"dvt{li % 2}", bufs=1,
                                       name=f"dvt{li}")
            e0b = b * kmax * P
            T = tpool.tile([P, kmax, t_elem_], cdt_, tag="T")
            co_v = comb_own[li][:].bitcast(cdt_)
            nc.gpsimd.dma_gather(
                T[:].bitcast(i32),
                co_v[:, t_off_:t_off_ + t_elem_].bitcast(i32),
                idxdl_t[:, e0b // 16:(e0b + kmax * P) // 16],
                kmax * P, kmax * P, t_elem_ * esz_ // 4,
                elem_step=cw_ * esz_ // 4, single_packet=False)
            Tf_ = T[:].bitcast(f32)
            nc.vector.tensor_copy(
                dvt_t[li][:, b, :, :], Tf_[:, :, sf0_ + H_:sf0_ + 2 * H_])

        # ==== phase 1 of layer 1: every core computes ALL blocks ====
        load_p1_weights(0)
        for g in range(nbf):
            phase1_block(0, g)

        for li, (fi, fo, H, C) in enumerate(LAYERS):
            pr = PR[li]
            kin = pr["kin"]
            last = li == 2
            cdt = pr["cdt"]
            esz = pr["esz"]
            cw, tail0, tsd = pr["cw"], pr["tail0"], pr["tsd"]
            t_elem, t_off, sf0 = pr["t_elem"], pr["t_off"], pr["sf0"]
            f_skip = pr["f_skip"]
            segs = pr["segs"]
            full = li == 0
            fold = not last         # exp folded into per-head masks

            def lhs_own(k, b):
                if li == 0:
                    return xTo_sb[:, b * P:(b + 1) * P]
                return yT_t[li - 1][:, k, b * P:(b + 1) * P]

            # -------- edge-phase weights --------
            linW_sb = wpool.tile([P, kin, f_skip], bf16, tag="linW")
            nc.sync.dma_start(
                linW_sb[:], W[li]["linW"].rearrange("(k p) f -> p k f", p=P))
            brow_sb = wpool.tile([1, f_skip], bf16, tag="brow")
            nc.sync.dma_start(brow_sb[:], W[li]["brow"][:])
            # next layer's phase-1 weights (load overlaps this edge phase)
            if li < 2:
                load_p1_weights(li + 1)

            # -------- scalar logit bound c --------
            reds, nblk = lctx[li]["reds"], lctx[li]["nblk"]
            csum = smallpool.tile([P, 1], f32, tag="csum")
            nc.vector.tensor_reduce(csum[:], reds[:, 0, :],
                                    axis=mybir.AxisListType.X, op=Alu.max)
            nc.vector.tensor_scalar_mul(csum[:], csum[:], 2.0)
            ct = psum_sm.tile([1, P], f32, tag="sm", name="ct")
            nc.tensor.transpose(ct[:], csum[:], ident[:])
            c1 = smallpool.tile([1, 1], f32, tag="c1")
            nc.vector.tensor_reduce(c1[:], ct[:], axis=mybir.AxisListType.X,
                                    op=Alu.max)
            pc = psum_sm.tile([P, 1], f32, tag="sm", name="pc")
            nc.tensor.matmul(pc[:], ones_f32[:], c1[:], start=True, stop=True)
            ncP = smallpool.tile([P, 1], f32, tag="cP")
            nc.scalar.activation(ncP[:], pc[:], Act.Copy, scale=-1.0)

            # -------- T-gathers: d[dst] tails (own dst rows) --------
            # For layers 2/3 these read the core-local comb_own with local
            # dst ids and run inside the AllGather window.
            if not full:
                for b in range(nb):
                    dvt_gather(li, b)
            dvt = dvt_t.get(li)

            # -------- edge phase per dst block --------
            if not last:
                yT_t[li] = yTpool.tile([P, fo // P, npc], bf16,
                                       tag=f"yT{li % 2}", name=f"yT_new{li}")
            yT_new = yT_t.get(li)
            comb_ap = comb_full[li][:].bitcast(cdt)
            sksegs = [(h0, min(512, f_skip - h0)) for h0 in range(0, f_skip, 512)]
            for b in range(nb):
                e0b = b * kmax * P
                if full:
                    # d[dst] tails for the whole block: one gather
                    T = tpool.tile([P, kmax, t_elem], cdt, tag="T")
                    nc.gpsimd.dma_gather(
                        T[:].bitcast(i32),
                        comb_ap[:, t_off:t_off + t_elem].bitcast(i32),
                        idxd_t[:, e0b // 16:(e0b + kmax * P) // 16],
                        kmax * P, kmax * P, t_elem * esz // 4,
                        elem_step=cw * esz // 4, single_packet=False)
                    Tf = T[:].bitcast(f32)

                # skip GEMM for this block (PE, overlaps gathers)
                ps = [psum_mm.tile([P, 512], f32, tag="mm", name=f"ps{si}")
                      for si in range(len(sksegs))]
                for si, (h0, hw_) in enumerate(sksegs):
                    for k in range(kin):
                        nc.tensor.matmul(
                            ps[si][:, 0:hw_], lhs_own(k, b),
                            linW_sb[:, k, h0:h0 + hw_],
                            start=(k == 0), stop=False)
                    nc.tensor.matmul(
                        ps[si][:, 0:hw_], ones_bf[:],
                        brow_sb[:, h0:h0 + hw_],
                        start=False, stop=True)

                pagg = [psum_agg.tile([P, 512], f32, tag="pagg",
                                      name=f"pagg{si}")
                        for si in range(len(segs))]
                pden = psum_sm.tile([P, H], f32, tag="sm", name="pden",
                                    padded_shape=[P, 512])
                Gs = []
                for (k0, kh) in pieces:
                    e0 = (b * kmax + k0) * P
                    n_idx = kh * P
                    G = gpool.tile([P, kq, cw], cdt, tag="G")
                    nc.gpsimd.dma_gather(
                        G[:, 0:kh, :].bitcast(i32), comb_ap[:, :].bitcast(i32),
                        idxs_t[:, e0 // 16:(e0 + n_idx) // 16],
                        n_idx, n_idx, cw * esz // 4,
                        elem_step=cw * esz // 4, single_packet=False)
                    Gs.append(G)
                hpb = 512 // C  # heads per psum bank
                for pi, (k0, kh) in enumerate(pieces):
                    G = Gs[pi]
                    # alternate the piece's mask work between Pool and DVE
                    veng = nc.gpsimd if pi % 2 == 0 else nc.vector
                    Sm = mpool.tile([P, kq, P], cdt, tag="Sm")
                    for k in range(kh):
                        gk = b * kmax + k0 + k
                        veng.tensor_scalar(
                            Sm[:, k, :], iot[:], dstloc_t[:, gk:gk + 1], None,
                            op0=Alu.is_equal)

                    # logits -> exp(leaky(s+d) - c)  (e - c <= 0 since c is
                    # the global max bound, so no overflow clamp is needed;
                    # -c is folded into the exp bias)
                    sv = G[:, 0:kh,
                           tail0:tail0 + 4 * H // esz].bitcast(f32)
                    dv = (Tf[:, k0:k0 + kh, sf0 + H:sf0 + 2 * H] if full
                          else dvt[:, b, k0:k0 + kh, :])
                    ee = epool.tile([P, kq, H], f32, tag="ee")
                    nc.vector.tensor_add(ee[:, 0:kh, :], sv, dv)
                    nc.vector.scalar_tensor_tensor(
                        ee[:, 0:kh, :], ee[:, 0:kh, :], NEG_SLOPE,
                        ee[:, 0:kh, :], op0=Alu.mult, op1=Alu.max)
                    ex = epool.tile([P, kq, H], bf16, tag="ex")
                    nc.scalar.activation(ex[:, 0:kh, :], ee[:, 0:kh, :],
                                         Act.Exp, bias=ncP[:, 0:1])

                    hh2 = H // 2
                    if not last:
                        # per-head masks Smh = Sm * ex, split by head halves
                        # across Pool and DVE to shorten the critical link
                        Smh = mpool.tile([P, kq, H, P], bf16, tag="Smh")
                        for eng, ha, hbnd in ((nc.gpsimd, 0, hh2),
                                              (nc.vector, hh2, H)):
                            nh_ = hbnd - ha
                            eng.tensor_mul(
                                Smh[:, 0:kh, ha:hbnd],
                                Sm[:, 0:kh].unsqueeze(2).broadcast_to(
                                    [P, kh, nh_, P]),
                                ex[:, 0:kh, ha:hbnd].unsqueeze(3).broadcast_to(
                                    [P, kh, nh_, P]))
                        for k in range(kh):
                            kk = k0 + k
                            st, sp = kk == 0, kk == kmax - 1
                            for hh in range(H):
                                si, off = divmod(hh * C, 512)
                                bank_last = min((si + 1) * hpb, H) - 1
                                nc.tensor.matmul(
                                    pagg[si][:, off:off + C],
                                    Smh[:, k, hh, :],
                                    G[:, k, hh * C:(hh + 1) * C],
                                    start=st and hh % hpb == 0,
                                    stop=sp and hh == bank_last)
                            nc.tensor.matmul(pden[:], Sm[:, k, :],
                                             ex[:, k, :], start=st, stop=sp)
                    else:
                        # L3: scale rows into bf16 (fo < H*P), keep raw mask
                        Gb = mpool.tile([P, kq, fo], bf16, tag="Gb",
                                        bufs=2)
                        Gbv = Gb[:, 0:kh].rearrange("p k (h c) -> p k h c",
                                                    h=H)
                        Ghv = G[:, 0:kh, 0:fo].rearrange(
                            "p k (h c) -> p k h c", h=H)
                        for eng, ha, hbnd in ((nc.gpsimd, 0, hh2),
                                              (nc.vector, hh2, H)):
                            nh_ = hbnd - ha
                            eng.tensor_mul(
                                Gbv[:, :, ha:hbnd], Ghv[:, :, ha:hbnd],
                                ex[:, 0:kh, ha:hbnd].unsqueeze(3).broadcast_to(
                                    [P, kh, nh_, C]))
                        for k in range(kh):
                            kk = k0 + k
                            st, sp = kk == 0, kk == kmax - 1
                            nc.tensor.matmul(pagg[0][:, 0:fo], Sm[:, k, :],
                                             Gb[:, k, :], start=st, stop=sp)
                            nc.tensor.matmul(pden[:], Sm[:, k, :],
                                             ex[:, k, :], start=st, stop=sp)

                # epilogue for block b
                rden = smallpool.tile([P, H], f32, tag="rden")
                nc.vector.reciprocal(rden[:], pden[:])
                yf = ypool.tile([P, fo], f32, tag="yf")
                if not last:
                    for si, (h0, hw_) in enumerate(segs):
                        nh = hw_ // C
                        hh0 = h0 // C
                        nc.vector.tensor_mul(
                            yf[:, h0:h0 + hw_].rearrange(
                                "p (h c) -> p h c", h=nh),
                            pagg[si][:, 0:hw_].rearrange(
                                "p (h c) -> p h c", h=nh),
                            rden[:, hh0:hh0 + nh].unsqueeze(2)
                                .broadcast_to([P, nh, C]))
                    for si, (h0, hw_) in enumerate(sksegs):
                        nc.vector.tensor_add(yf[:, h0:h0 + hw_],
                                             yf[:, h0:h0 + hw_],
                                             ps[si][:, 0:hw_])
                    # ELU: y = max(yf,0) + exp(min(yf,0)) - 1
                    mn = ypool.tile([P, fo], f32, tag="mn", bufs=1)
                    nc.gpsimd.tensor_scalar_min(mn[:], yf[:], 0.0)
                    nc.scalar.activation(mn[:], mn[:], Act.Exp)
                    nc.vector.scalar_tensor_tensor(
                        yf[:], yf[:], 0.0, mn[:], op0=Alu.max, op1=Alu.add)
                    nc.scalar.activation(yf[:], yf[:], Act.Copy, bias=-1.0)
                    # transpose into yT_new (4 transposes per psum bank,
                    # one Act copy per bank)
                    for j0 in range(0, fo // P, 4):
                        jn = min(4, fo // P - j0)
                        pt = psum_sm.tile([P, 4, P], f32, tag="sm", name="pt",
                                          padded_shape=[P, 4, P])
                        for j in range(jn):
                            nc.tensor.transpose(
                                pt[:, j, :], yf[:, (j0 + j) * P:(j0 + j + 1) * P],
                                ident[:])
                        nc.scalar.activation(
                            yT_new[:, j0:j0 + jn, b * P:(b + 1) * P],
                            pt[:, 0:jn, :], Act.Copy)
                    # next layer's phase 1 for this block (pipelined so the
                    # AllGather input is ready as soon as the loop ends)
                    phase1_block(li + 1, b)
                else:
                    for si, (h0, hw_) in enumerate(segs):
                        nh = hw_ // C
                        hh0 = h0 // C
                        nc.vector.tensor_mul(
                            yf[:, h0:h0 + hw_].rearrange(
                                "p (h c) -> p h c", h=nh),
                            pagg[si][:, 0:hw_].rearrange(
                                "p (h c) -> p h c", h=nh),
                            rden[:, hh0:hh0 + nh].unsqueeze(2)
                                .broadcast_to([P, nh, C]))
                    # mean over heads + skip
                    yo = ypool.tile([P, OUT_DIM], f32, tag="yo")
                    nc.vector.tensor_reduce(
                        yo[:], yf[:].rearrange("p (h c) -> p c h", h=H),
                        axis=mybir.AxisListType.X, op=Alu.add)
                    nc.vector.tensor_scalar_mul(yo[:], yo[:], 1.0 / H)
                    nc.vector.tensor_add(yo[:], yo[:], ps[0][:, 0:OUT_DIM])
                    nc.sync.dma_start(out_dram[b * P:(b + 1) * P, :], yo[:])

            # -------- AllGather next layer's combined rows --------
            if li < 2:
                nc.gpsimd.collective_compute(
                    "AllGather", Alu.bypass, replica_groups=replica_groups,
                    ins=[comb_own[li + 1][:]], outs=[comb_full[li + 1][:]])

    nc.compile()
    return nc


# ---------------------------------------------------------------------------
# host wrapper
# ---------------------------------------------------------------------------

@functools.lru_cache(maxsize=2)
def _cached_program(n_pad, kmax):
    return build_program(n_pad, kmax)


def make_in_maps(x, edge_index, weights):
    """weights: list of 3 dicts with keys W, linW, brow, aS, aD (numpy f32)."""
    n = x.shape[0]
    n_pad = cdiv(n, NCORES * P) * NCORES * P
    npc = n_pad // NCORES
    nb = npc // P

    g = _prep_graph(edge_index, n_pad)

    x_pad = np.zeros((n_pad, x.shape[1]), np.float32)
    x_pad[:n] = np.asarray(x, np.float32)
    xT_all = np.ascontiguousarray(x_pad.T.astype(BF16))

    layer_w = []
    for li, lw in enumerate(weights):
        Wf = np.asarray(lw["W"], np.float64)
        aS = np.asarray(lw["aS"], np.float64)   # [H, C]
        aD = np.asarray(lw["aD"], np.float64)
        H, C = aS.shape
        fo = H * C
        # WA[k, h] = sum_c W[k, h*C+c] * a[h, c]
        Wr = Wf.reshape(-1, H, C)
        WAs = np.einsum("khc,hc->kh", Wr, aS)
        WAd = np.einsum("khc,hc->kh", Wr, aD)
        WA = np.concatenate([WAs, WAd], axis=1)  # [fi, 2H]
        layer_w.append(dict(
            W=np.ascontiguousarray(Wf.astype(BF16)),
            linW=np.ascontiguousarray(
                np.asarray(lw["linW"], np.float32).astype(BF16)),
            brow=np.ascontiguousarray(
                np.asarray(lw["brow"], np.float32).astype(BF16)[None, :]),
            WA=np.ascontiguousarray(WA.astype(BF16)),
        ))

    in_maps = []
    for c in range(NCORES):
        blo, bhi = c * nb, (c + 1) * nb
        nbc = bhi - blo
        kmax = g["kmax"]
        m = dict(
            xT=xT_all,
            xT_own=np.ascontiguousarray(xT_all[:, c * npc:(c + 1) * npc]),
            idx_src=_wrap_idx(g["src_pad"][blo:bhi].reshape(-1)),
            idx_dst=_wrap_idx(g["dst_pad"][blo:bhi].reshape(-1)),
            idx_dstl=_wrap_idx(np.maximum(
                g["dst_pad"][blo:bhi].reshape(-1) - c * npc, 0)),
            dstloc=np.ascontiguousarray(
                g["dl"][blo:bhi].transpose(1, 0, 2).reshape(P, nbc * kmax)
                .astype(np.float32)),
        )
        for li, lw in enumerate(layer_w):
            for key in ("W", "linW", "brow", "WA"):
                m[f"{key}{li}"] = lw[key]
        in_maps.append(m)
    return in_maps, g, n_pad


def _weights_from_kwargs(W1, a1_src, a1_dst, b1, lin1_W, lin1_b,
                         W2, a2_src, a2_dst, b2, lin2_W, lin2_b,
                         W3, a3_src, a3_dst, b3, lin3_W, lin3_b):
    return [
        dict(W=W1, linW=lin1_W, brow=np.asarray(b1) + np.asarray(lin1_b),
             aS=a1_src, aD=a1_dst),
        dict(W=W2, linW=lin2_W, brow=np.asarray(b2) + np.asarray(lin2_b),
             aS=a2_src, aD=a2_dst),
        dict(W=W3, linW=lin3_W, brow=np.asarray(b3) + np.asarray(lin3_b),
             aS=a3_src, aD=a3_dst),
    ]


def run_gat(inputs, trace=False, **run_kwargs):
    from concourse.bass_utils import run_bass_kernel_spmd

    kw = {k: inputs[k] for k in (
        "W1", "a1_src", "a1_dst", "b1", "lin1_W", "lin1_b",
        "W2", "a2_src", "a2_dst", "b2", "lin2_W", "lin2_b",
        "W3", "a3_src", "a3_dst", "b3", "lin3_W", "lin3_b")}
    weights = _weights_from_kwargs(**kw)
    x, edge_index = inputs["x"], inputs["edge_index"]
    in_maps, g, n_pad = make_in_maps(x, edge_index, weights)
    nc = _cached_program(n_pad, g["kmax"])
    res = run_bass_kernel_spmd(nc, in_maps, list(range(NCORES)),
                               trace=trace, **run_kwargs)
    out = np.concatenate([res.results[c]["out"] for c in range(NCORES)],
                         axis=0)
    n = x.shape[0]
    return np.ascontiguousarray(out[:n]).astype(np.float32), res


def kernel(**inputs):
    return run_gat(inputs)[0]

